# revision 1
# baseline (speedup 1.0000x reference)
"""2-layer GCN (PyG GCNConv x2 + ReLU) on 8 Trainium2 NeuronCores.

Math (per layer, A = adjacency from edge_index, D = deg(dst)+1 with self loops):
    h   = x @ W
    out = relu(dis * (A @ (dis*h) + dis*h) + b),   dis = D^{-1/2}

Sharding: nodes are packed into 128-node "blocks" balanced by in-degree
(snake assignment over degree-sorted nodes).  Each of the 8 cores owns
NB blocks.  Per layer: each core computes h' = (x@W)*dis for its nodes,
the h' tables are AllGathered (bf16), and each core aggregates messages
for its own dst blocks by indirect-DMA gathering h'[src] rows and
summing them with one-hot matmuls accumulated in PSUM.
"""

import math

import ml_dtypes
import numpy as np

import concourse.bass as bass
import concourse.mybir as mybir
import concourse.tile as tile
from concourse.bass_utils import run_bass_kernel_spmd
from concourse.masks import make_identity
from concourse.vector_clock import ScopedClock

P = 128
NCORES = 8
F32 = mybir.dt.float32
BF16 = mybir.dt.bfloat16
I32 = mybir.dt.int32
PAD_LANE = 1000.0  # dst-lane sentinel for padding edge slots (one-hot = 0)


def _patched_drain_and_barrier(self, tick_clock, wait_clock):
    # This walrus build rejects >1 sem wait on TPB_CTRL (Drain) instructions.
    # Spill the tile-epilogue drain waits onto extra single-wait drains.
    drain_inst = self.nc.sync.drain()
    wait_clock.add_sem_waits(
        drain_inst.ins, ScopedClock({None: tick_clock.global_clock})
    )
    si = drain_inst.ins.sync_info
    waits = list(si.on_wait)
    if len(waits) > 1:
        while len(si.on_wait):
            si.on_wait.pop()
        si.on_wait.append(waits[0])
        for w in waits[1:]:
            d2 = self.nc.sync.drain(fusable=False)
            si2 = d2.ins.sync_info
            if si2 is None:
                d2.ins.sync_info = mybir.SyncInfo(on_wait=[w], on_update=[])
            else:
                si2.on_wait.append(w)
    self.nc.all_engine_barrier()
    popped = self.nc._tile_sem_poison_stack.pop()
    assert popped is self._sem_poison
    self.nc.clear_and_free_semaphores(list(self.sems.allocated().values()))
    self.nc.all_engine_barrier()


tile.TileContext._drain_and_barrier = _patched_drain_and_barrier


def _spill_waits(nc, max_waits=1):
    """This walrus build accepts at most one sync wait per instruction.
    Move extra waits onto dedicated single-wait NoOps ahead of the
    instruction on the same engine (engines execute in program order)."""
    n = 0
    for f in nc.m.functions:
        for blk in f.blocks:
            il = blk.instructions
            out = []
            for inst in il:
                si = inst.sync_info
                if si is not None and len(si.on_wait) > max_waits:
                    waits = list(si.on_wait)
                    while len(si.on_wait):
                        si.on_wait.pop()
                    for w in waits[:max_waits]:
                        si.on_wait.append(w)
                    for w in waits[max_waits:]:
                        nop = mybir.InstNoOp(
                            name=f"waitspill-{n}",
                            sync_info=mybir.SyncInfo(on_wait=[w], on_update=[]),
                            bass_nofuse=True,
                            engine=inst.engine,
                        )
                        n += 1
                        out.append(nop)
                out.append(inst)
            blk.instructions = out
    return n


def _build_program(NB, CPB, IN_CH, HID, OUT_CH, has_b1, has_b2):
    """One SPMD program; per-core data comes via input tensors."""
    NPC = NB * P  # nodes per core
    NTOT = NPC * NCORES  # rows in the allgathered tables
    KT = IN_CH // P  # k-tiles for the layer-1 dense matmul
    assert IN_CH % P == 0 and HID <= 512 and OUT_CH <= 512

    nc = bass.Bass()
    xT = nc.dram_tensor("xT", [IN_CH, NPC], F32, kind="ExternalInput")
    W1 = nc.dram_tensor("W1", [IN_CH, HID], F32, kind="ExternalInput")
    W2 = nc.dram_tensor("W2", [HID, OUT_CH], F32, kind="ExternalInput")
    b1bc = nc.dram_tensor("b1bc", [P, HID], F32, kind="ExternalInput")
    b2bc = nc.dram_tensor("b2bc", [P, OUT_CH], F32, kind="ExternalInput")
    disT = nc.dram_tensor("disT", [P, NB], F32, kind="ExternalInput")
    esrc = nc.dram_tensor("esrc", [P, NB * CPB], I32, kind="ExternalInput")
    dstl = nc.dram_tensor("dstl", [P, NB * CPB], F32, kind="ExternalInput")
    outY = nc.dram_tensor("outY", [NPC, OUT_CH], F32, kind="ExternalOutput")

    h1s = nc.dram_tensor("h1s", [NPC, HID], BF16)
    h1f = nc.dram_tensor("h1f", [NTOT, HID], BF16)
    h2s = nc.dram_tensor("h2s", [NPC, OUT_CH], BF16)
    h2f = nc.dram_tensor("h2f", [NTOT, OUT_CH], BF16)

    rg = [list(range(NCORES))]
    RELU = mybir.ActivationFunctionType.Relu
    ADD = mybir.AluOpType.add
    ISEQ = mybir.AluOpType.is_equal

    with tile.TileContext(nc) as tc:
        with tc.tile_pool(name="const", bufs=1) as cst:
            w1sb = cst.tile([P, KT * HID], F32)
            for k in range(KT):
                nc.sync.dma_start(
                    out=w1sb[:, k * HID : (k + 1) * HID], in_=W1[k * P : (k + 1) * P, :]
                )
            w2sb = cst.tile([P, OUT_CH], F32)
            nc.sync.dma_start(out=w2sb[:], in_=W2[:, :])
            b1sb = cst.tile([P, HID], F32)
            nc.sync.dma_start(out=b1sb[:], in_=b1bc[:, :])
            b2sb = cst.tile([P, OUT_CH], F32)
            nc.sync.dma_start(out=b2sb[:], in_=b2bc[:, :])
            dissb = cst.tile([P, NB], F32)
            nc.sync.dma_start(out=dissb[:], in_=disT[:, :])
            esrcsb = cst.tile([P, NB * CPB], I32)
            nc.sync.dma_start(out=esrcsb[:], in_=esrc[:, :])
            dstlsb = cst.tile([P, NB * CPB], F32)
            nc.sync.dma_start(out=dstlsb[:], in_=dstl[:, :])
            iotasb = cst.tile([P, P], BF16)
            nc.gpsimd.iota(
                iotasb[:],
                pattern=[[1, P]],
                base=0,
                channel_multiplier=0,
                allow_small_or_imprecise_dtypes=True,
            )
            idsb = cst.tile([P, P], F32)
            make_identity(nc, idsb[:])
            h1p = cst.tile([P, NB * HID], F32)  # h' shard, layer 1 (f32)
            h2p = cst.tile([P, NB * OUT_CH], F32)  # h' shard, layer 2 (f32)

            # ---- Phase A: h1' = (x @ W1) * dis  (per block)
            with (
                tc.tile_pool(name="pa", bufs=3) as pa,
                tc.tile_pool(name="pap", bufs=2, space="PSUM") as pap,
            ):
                for b in range(NB):
                    xt = pa.tile([P, KT * P], F32, tag="xt")
                    for k in range(KT):
                        nc.sync.dma_start(
                            out=xt[:, k * P : (k + 1) * P],
                            in_=xT[k * P : (k + 1) * P, b * P : (b + 1) * P],
                        )
                    ps0 = pap.tile([P, HID], F32, tag="ps0")
                    for k in range(KT):
                        nc.tensor.matmul(
                            ps0[:],
                            lhsT=xt[:, k * P : (k + 1) * P],
                            rhs=w1sb[:, k * HID : (k + 1) * HID],
                            start=(k == 0),
                            stop=(k == KT - 1),
                        )
                    nc.vector.tensor_scalar_mul(
                        h1p[:, b * HID : (b + 1) * HID], ps0[:], dissb[:, b : b + 1]
                    )

            # ---- Phase B: shard -> DRAM (cast bf16), AllGather
            nc.gpsimd.dma_start(
                out=h1s[:, :].rearrange("(b p) f -> p b f", p=P),
                in_=h1p[:].rearrange("p (b f) -> p b f", f=HID),
            )
            nc.gpsimd.collective_compute(
                "AllGather",
                mybir.AluOpType.bypass,
                replica_groups=rg,
                ins=[h1s[:, :]],
                outs=[h1f[:, :]],
            )

            # ---- Phase C: aggregate layer 1, layer-1 epilogue, h2' = (out1@W2)*dis
            with (
                tc.tile_pool(name="pc", bufs=3) as pc,
                tc.tile_pool(name="pcm", bufs=4) as pcm,
                tc.tile_pool(name="pcp", bufs=2, space="PSUM") as pcp,
                tc.tile_pool(name="pcq", bufs=2, space="PSUM") as pcq,
            ):
                for b in range(NB):
                    g1 = pc.tile([P, CPB * HID], BF16, tag="g1")
                    for c in range(CPB):
                        col = b * CPB + c
                        nc.gpsimd.indirect_dma_start(
                            out=g1[:, c * HID : (c + 1) * HID],
                            out_offset=None,
                            in_=h1f[:, :],
                            in_offset=bass.IndirectOffsetOnAxis(
                                ap=esrcsb[:, col : col + 1], axis=0
                            ),
                        )
                    ps1 = pcp.tile([P, HID], F32, tag="ps1")
                    for c in range(CPB):
                        m = pcm.tile([P, P], BF16, tag="m")
                        col = b * CPB + c
                        nc.vector.tensor_scalar(
                            m[:], iotasb[:], dstlsb[:, col : col + 1], None, ISEQ
                        )
                        nc.tensor.matmul(
                            ps1[:],
                            lhsT=m[:],
                            rhs=g1[:, c * HID : (c + 1) * HID],
                            start=(c == 0),
                            stop=(c == CPB - 1),
                        )
                    t0 = pc.tile([P, HID], F32, tag="t0")
                    nc.vector.tensor_tensor(
                        t0[:], ps1[:], h1p[:, b * HID : (b + 1) * HID], op=ADD
                    )
                    o1 = pc.tile([P, HID], F32, tag="o1")
                    if has_b1:
                        nc.vector.tensor_scalar_mul(t0[:], t0[:], dissb[:, b : b + 1])
                        nc.vector.tensor_tensor(t0[:], t0[:], b1sb[:], op=ADD)
                        nc.scalar.activation(o1[:], t0[:], RELU)
                    else:
                        nc.scalar.activation(
                            o1[:], t0[:], RELU, scale=dissb[:, b : b + 1]
                        )
                    pst = pcq.tile([P, HID], F32, tag="pst")
                    nc.tensor.transpose(out=pst[:], in_=o1[:], identity=idsb[:])
                    o1t = pc.tile([P, HID], F32, tag="o1t")
                    nc.scalar.copy(out=o1t[:], in_=pst[:])
                    ps2 = pcq.tile([P, OUT_CH], F32, tag="ps2")
                    nc.tensor.matmul(
                        ps2[:], lhsT=o1t[:], rhs=w2sb[:], start=True, stop=True
                    )
                    nc.vector.tensor_scalar_mul(
                        h2p[:, b * OUT_CH : (b + 1) * OUT_CH],
                        ps2[:],
                        dissb[:, b : b + 1],
                    )

            # ---- Phase D: shard -> DRAM (cast bf16), AllGather
            nc.gpsimd.dma_start(
                out=h2s[:, :].rearrange("(b p) f -> p b f", p=P),
                in_=h2p[:].rearrange("p (b f) -> p b f", f=OUT_CH),
            )
            nc.gpsimd.collective_compute(
                "AllGather",
                mybir.AluOpType.bypass,
                replica_groups=rg,
                ins=[h2s[:, :]],
                outs=[h2f[:, :]],
            )

            # ---- Phase E: aggregate layer 2, final epilogue, write output
            with (
                tc.tile_pool(name="pe", bufs=3) as pe,
                tc.tile_pool(name="pem", bufs=4) as pem,
                tc.tile_pool(name="pep", bufs=2, space="PSUM") as pep,
            ):
                for b in range(NB):
                    g2 = pe.tile([P, CPB * OUT_CH], BF16, tag="g2")
                    for c in range(CPB):
                        col = b * CPB + c
                        nc.gpsimd.indirect_dma_start(
                            out=g2[:, c * OUT_CH : (c + 1) * OUT_CH],
                            out_offset=None,
                            in_=h2f[:, :],
                            in_offset=bass.IndirectOffsetOnAxis(
                                ap=esrcsb[:, col : col + 1], axis=0
                            ),
                        )
                    ps3 = pep.tile([P, OUT_CH], F32, tag="ps3")
                    for c in range(CPB):
                        m2 = pem.tile([P, P], BF16, tag="m2")
                        col = b * CPB + c
                        nc.vector.tensor_scalar(
                            m2[:], iotasb[:], dstlsb[:, col : col + 1], None, ISEQ
                        )
                        nc.tensor.matmul(
                            ps3[:],
                            lhsT=m2[:],
                            rhs=g2[:, c * OUT_CH : (c + 1) * OUT_CH],
                            start=(c == 0),
                            stop=(c == CPB - 1),
                        )
                    t2 = pe.tile([P, OUT_CH], F32, tag="t2")
                    nc.vector.tensor_tensor(
                        t2[:], ps3[:], h2p[:, b * OUT_CH : (b + 1) * OUT_CH], op=ADD
                    )
                    o2 = pe.tile([P, OUT_CH], F32, tag="o2")
                    if has_b2:
                        nc.vector.tensor_scalar_mul(t2[:], t2[:], dissb[:, b : b + 1])
                        nc.vector.tensor_tensor(t2[:], t2[:], b2sb[:], op=ADD)
                        nc.scalar.activation(o2[:], t2[:], RELU)
                    else:
                        nc.scalar.activation(
                            o2[:], t2[:], RELU, scale=dissb[:, b : b + 1]
                        )
                    nc.sync.dma_start(out=outY[b * P : (b + 1) * P, :], in_=o2[:])

    _spill_waits(nc)
    return nc


def _prepare(x, src, dst):
    """Host-side sharding: degree-balanced node->block assignment + edge slots."""
    N = x.shape[0]
    E = src.shape[0]
    NB = int(math.ceil(N / (NCORES * P)))  # blocks per core
    TB = NB * NCORES  # total blocks
    NPC = NB * P
    NTOT = NPC * NCORES

    indeg = np.bincount(dst, minlength=N).astype(np.int64)
    dis = (1.0 / np.sqrt(indeg.astype(np.float32) + 1.0)).astype(np.float32)

    # Snake assignment of degree-sorted nodes over TB blocks -> balanced
    # per-block edge counts; round r = lane r (<=128 rounds by construction).
    order = np.argsort(-indeg, kind="stable")
    i = np.arange(N)
    rnd = i // TB
    pos = i % TB
    blk_i = np.where(rnd % 2 == 0, pos, TB - 1 - pos)
    assert rnd.max() < P
    gid_of = np.empty(N, np.int64)
    gid_of[order] = blk_i * P + rnd
    node_of_gid = np.full(NTOT, -1, np.int64)
    node_of_gid[gid_of] = np.arange(N)

    # Edge slots: group edges by dst block; slot (chunk, lane) within block.
    gdst = gid_of[dst]
    eblk = gdst >> 7
    eord = np.argsort(eblk, kind="stable")
    eblk_s = eblk[eord]
    counts = np.bincount(eblk_s, minlength=TB)
    CPB = int(math.ceil(counts.max() / P))
    ofs = np.zeros(TB + 1, np.int64)
    np.cumsum(counts, out=ofs[1:])
    pos_in_blk = np.arange(E) - ofs[eblk_s]
    chunk = pos_in_blk // P
    lane = pos_in_blk % P

    esrc_full = np.zeros((TB, CPB, P), np.int32)
    dstl_full = np.full((TB, CPB, P), PAD_LANE, np.float32)
    esrc_full[eblk_s, chunk, lane] = gid_of[src[eord]].astype(np.int32)
    dstl_full[eblk_s, chunk, lane] = (gdst[eord] & 127).astype(np.float32)

    return dict(
        NB=NB, CPB=CPB, NPC=NPC, NTOT=NTOT,
        dis=dis, gid_of=gid_of, node_of_gid=node_of_gid,
        esrc_full=esrc_full, dstl_full=dstl_full,
    )


def kernel(x, edge_index, W1, b1, W2, b2):
    x = np.ascontiguousarray(np.asarray(x, dtype=np.float32))
    W1 = np.ascontiguousarray(np.asarray(W1, dtype=np.float32))
    W2 = np.ascontiguousarray(np.asarray(W2, dtype=np.float32))
    b1 = np.asarray(b1, dtype=np.float32)
    b2 = np.asarray(b2, dtype=np.float32)
    src = np.asarray(edge_index[0]).astype(np.int64)
    dst = np.asarray(edge_index[1]).astype(np.int64)

    N, IN_CH = x.shape
    HID = W1.shape[1]
    OUT_CH = W2.shape[1]
    pr = _prepare(x, src, dst)
    NB, CPB, NPC = pr["NB"], pr["CPB"], pr["NPC"]
    node_of_gid = pr["node_of_gid"]
    dis = pr["dis"]

    has_b1 = bool(np.any(b1))
    has_b2 = bool(np.any(b2))
    nc = _build_program(NB, CPB, IN_CH, HID, OUT_CH, has_b1, has_b2)

    b1bc = np.ascontiguousarray(np.broadcast_to(b1, (P, HID)))
    b2bc = np.ascontiguousarray(np.broadcast_to(b2, (P, OUT_CH)))

    in_maps = []
    for c in range(NCORES):
        slots = node_of_gid[c * NPC : (c + 1) * NPC]  # [NPC] orig node or -1
        valid = slots >= 0
        xs = np.zeros((NPC, IN_CH), np.float32)
        xs[valid] = x[slots[valid]]
        xTc = np.ascontiguousarray(xs.T)
        disc = np.ones(NPC, np.float32)
        disc[valid] = dis[slots[valid]]
        disTc = np.ascontiguousarray(disc.reshape(NB, P).T)
        # [block, chunk, lane] -> [lane, block*CPB + chunk]
        esrcc = np.ascontiguousarray(
            pr["esrc_full"][c * NB : (c + 1) * NB].transpose(2, 0, 1).reshape(P, -1)
        )
        dstlc = np.ascontiguousarray(
            pr["dstl_full"][c * NB : (c + 1) * NB]
            .transpose(2, 0, 1)
            .reshape(P, -1)
        )
        in_maps.append(
            {
                "xT": xTc,
                "W1": W1,
                "W2": W2,
                "b1bc": b1bc,
                "b2bc": b2bc,
                "disT": disTc,
                "esrc": esrcc,
                "dstl": dstlc,
            }
        )

    res = run_bass_kernel_spmd(nc, in_maps, core_ids=list(range(NCORES)))
    global _last_results, _last_nc
    _last_results = res
    _last_nc = nc

    out = np.empty((N, OUT_CH), np.float32)
    for c in range(NCORES):
        oc = res.results[c]["outY"]
        slots = node_of_gid[c * NPC : (c + 1) * NPC]
        valid = slots >= 0
        out[slots[valid]] = oc[valid]
    return out



# revision 8
# speedup vs baseline: 3.0284x; 3.0284x over previous
"""2-layer GCN (PyG GCNConv x2 + ReLU) on 8 Trainium2 NeuronCores.

Math per layer (A from edge_index, deg = indeg(dst)+1, dis = deg^-1/2):
    out[d] = relu( dis_d * ( sum_{e: s->d} dis_s*h[s] + dis_d*h[d] ) @ W + b )

Strategy (v2 — replaces per-chunk indirect DMAs of the first version):
  * Layer 1 is computed aggregate-then-transform: every core holds a full
    replica of xdis = x*dis (bf16) in its HBM, so there is NO layer-1
    collective at all.  Each core aggregates Z = sum xdis[src] for its own
    destination blocks with large dma_gather calls (custom SWDGE gather,
    ~1us fixed cost amortized over thousands of rows) + one-hot-mask
    matmuls, then applies W1 (and W2, producing the layer-2 message table
    h2 = dis*(relu(.)@W2)) densely per 128-node block.
  * One AllGather shares the h2 tables (padded to 128 cols so gather rows
    are 256B), then layer 2 aggregates the same way.
  * Self-loops are folded in as ordinary edges (src == dst).
  * dma_gather uses int16 indices, so the node table is split in 4 row
    slices of <=32768 and edge slots are grouped by src-slice.  The slot
    schedule (segment sizes, chunk/block spans) is made identical across
    cores by padding each (block, slice) segment to the max over cores,
    keeping the SPMD program uniform; only tensor data differs per core.
"""

import math

import ml_dtypes
import numpy as np

import concourse.bass as bass
import concourse.mybir as mybir
import concourse.tile as tile
from concourse import library_config
from concourse.bass_utils import run_bass_kernel_spmd
from concourse.masks import make_identity
from concourse.vector_clock import ScopedClock

P = 128
NCORES = 8
PAD_LANE = 1000.0  # dst-lane sentinel for padding edge slots (one-hot = 0)
QS = 32768  # dma_gather int16 index range -> table row-slice size
GRP = 7  # dst blocks per gather group

F32 = mybir.dt.float32
BF16 = mybir.dt.bfloat16
I16 = mybir.dt.int16


def _patched_drain_and_barrier(self, tick_clock, wait_clock):
    # This walrus build rejects >1 sem wait on TPB_CTRL (Drain) instructions.
    # Spill the tile-epilogue drain waits onto extra single-wait drains.
    drain_inst = self.nc.sync.drain()
    wait_clock.add_sem_waits(
        drain_inst.ins, ScopedClock({None: tick_clock.global_clock})
    )
    si = drain_inst.ins.sync_info
    waits = list(si.on_wait)
    if len(waits) > 1:
        while len(si.on_wait):
            si.on_wait.pop()
        si.on_wait.append(waits[0])
        for w in waits[1:]:
            d2 = self.nc.sync.drain(fusable=False)
            si2 = d2.ins.sync_info
            if si2 is None:
                d2.ins.sync_info = mybir.SyncInfo(on_wait=[w], on_update=[])
            else:
                si2.on_wait.append(w)
    self.nc.all_engine_barrier()
    popped = self.nc._tile_sem_poison_stack.pop()
    assert popped is self._sem_poison
    self.nc.clear_and_free_semaphores(list(self.sems.allocated().values()))
    self.nc.all_engine_barrier()


tile.TileContext._drain_and_barrier = _patched_drain_and_barrier


def _spill_waits(nc, max_waits=1):
    """This walrus build accepts at most one sync wait per instruction.
    Move extra waits onto dedicated single-wait NoOps ahead of the
    instruction on the same engine (engines execute in program order)."""
    n = 0
    for f in nc.m.functions:
        for blk in f.blocks:
            il = blk.instructions
            out = []
            for inst in il:
                si = inst.sync_info
                if si is not None and len(si.on_wait) > max_waits:
                    waits = list(si.on_wait)
                    while len(si.on_wait):
                        si.on_wait.pop()
                    for w in waits[:max_waits]:
                        si.on_wait.append(w)
                    for w in waits[max_waits:]:
                        nop = mybir.InstNoOp(
                            name=f"waitspill-{n}",
                            sync_info=mybir.SyncInfo(on_wait=[w], on_update=[]),
                            bass_nofuse=True,
                            engine=inst.engine,
                        )
                        n += 1
                        out.append(nop)
                out.append(inst)
            blk.instructions = out
    return n


# --------------------------------------------------------------------------
# Host-side schedule construction
# --------------------------------------------------------------------------


class _Sched:
    pass


def _prepare(N, src, dst):
    """Node->gid assignment, slot buckets, and the (core-uniform) gather /
    column schedule."""
    s = _Sched()
    NB = int(math.ceil(N / (NCORES * P)))
    TB = NB * NCORES
    NPC = NB * P
    NTOT = TB * P
    NGRP = int(math.ceil(NB / GRP))
    assert NGRP * GRP == NB, (NB, NGRP)
    NQ = int(math.ceil(NTOT / QS))
    qbase = [q * QS for q in range(NQ)]
    qrows = [min(QS, NTOT - q * QS) for q in range(NQ)]

    E = src.shape[0]
    indeg = np.bincount(dst, minlength=N).astype(np.int64)
    dis = (1.0 / np.sqrt(indeg.astype(np.float64) + 1.0)).astype(np.float32)

    # snake assignment of degree-sorted nodes over TB blocks
    order = np.argsort(-indeg, kind="stable")
    i = np.arange(N)
    rnd = i // TB
    pos = i % TB
    blk_i = np.where(rnd % 2 == 0, pos, TB - 1 - pos)
    assert rnd.max() < P
    gid_of = np.empty(N, np.int64)
    gid_of[order] = blk_i * P + rnd
    node_of_gid = np.full(NTOT, -1, np.int64)
    node_of_gid[gid_of] = np.arange(N)

    # slots = edges + self-loops, bucketed by (dst block, src slice)
    all_src = np.concatenate([gid_of[src], gid_of])
    all_dst = np.concatenate([gid_of[dst], gid_of])
    sblk = all_dst >> 7
    slane = (all_dst & 127).astype(np.float32)
    sq = all_src // QS
    key = sblk * NQ + sq
    ord2 = np.argsort(key, kind="stable")
    k_src = all_src[ord2]
    k_lane = slane[ord2]
    cnt = np.bincount(key, minlength=TB * NQ).reshape(TB, NQ)
    offs = np.zeros(TB * NQ + 1, np.int64)
    np.cumsum(cnt.reshape(-1), out=offs[1:])

    # global block -> (core, position): sorted by total slots, snake
    btot = cnt.sum(1)
    bord = np.argsort(-btot, kind="stable")
    block_of = np.empty((NCORES, NB), np.int64)
    for p in range(NB):
        row = bord[p * NCORES : (p + 1) * NCORES]
        if p % 2:
            row = row[::-1]
        block_of[:, p] = row

    # renumber into PHYSICAL gid space: pgid = (core*NB + position)*128 + lane.
    # The AllGathered h2f table is laid out in physical order, so gather
    # indices, slice bucketing, the xdis table, and outputs all use pgid.
    phys_pos = np.empty(TB, np.int64)
    for c in range(NCORES):
        for p in range(NB):
            phys_pos[block_of[c, p]] = c * NB + p
    pgid_of = phys_pos[gid_of >> 7] * P + (gid_of & 127)
    gid_of = pgid_of
    node_of_gid = np.full(NTOT, -1, np.int64)
    node_of_gid[gid_of] = np.arange(N)

    # redo slot bucketing in physical space
    all_src = np.concatenate([gid_of[src], gid_of])
    all_dst = np.concatenate([gid_of[dst], gid_of])
    sblk = all_dst >> 7
    slane = (all_dst & 127).astype(np.float32)
    sq = all_src // QS
    key = sblk * NQ + sq
    ord2 = np.argsort(key, kind="stable")
    k_src = all_src[ord2]
    k_lane = slane[ord2]
    cnt = np.bincount(key, minlength=TB * NQ).reshape(TB, NQ)
    offs = np.zeros(TB * NQ + 1, np.int64)
    np.cumsum(cnt.reshape(-1), out=offs[1:])
    block_of = (np.arange(NCORES)[:, None] * NB) + np.arange(NB)[None, :]

    # uniform per-(position, slice) segment lengths
    seg_len = cnt[block_of, :].max(axis=0)  # [NB, NQ]

    # per-group call sizes + column schedule (shared by both layers)
    nch = np.zeros((NGRP, NQ), np.int64)
    oq = np.zeros((NGRP, NQ), np.int64)  # chunk offset of call q in group tile
    groups = []  # per group: list of (p, [(q, ci, t0, t1, start, stop)])
    for g in range(NGRP):
        ps = list(range(g * GRP, (g + 1) * GRP))
        for q in range(NQ):
            L = int(seg_len[ps, q].sum())
            nch[g, q] = (L + P - 1) // P
        oq[g] = np.concatenate([[0], np.cumsum(nch[g])[:-1]])
        blocks = []
        for bi, p in enumerate(ps):
            cols = []
            for q in range(NQ):
                t0 = int(seg_len[ps[:bi], q].sum())
                t1 = t0 + int(seg_len[p, q])
                if t1 == t0:
                    continue
                for ci in range(t0 // P, (t1 - 1) // P + 1):
                    cols.append((q, ci, t0, t1))
            assert cols
            blocks.append((p, cols))
        groups.append(blocks)
    NCHG = int(nch.sum(1).max())
    TOTCOL = sum(len(c) for blks in groups for _, c in blks)

    # int16 idx column offsets per call
    icol = np.zeros((NGRP, NQ), np.int64)
    run = 0
    for g in range(NGRP):
        for q in range(NQ):
            icol[g, q] = run
            run += int(nch[g, q]) * (P // 16)
    TOTICOL = int(run)

    s.NB, s.TB, s.NPC, s.NTOT, s.NGRP, s.NQ = NB, TB, NPC, NTOT, NGRP, NQ
    s.qbase, s.qrows = qbase, qrows
    s.dis, s.gid_of, s.node_of_gid = dis, gid_of, node_of_gid
    s.k_src, s.k_lane, s.cnt, s.offs = k_src, k_lane, cnt, offs
    s.block_of, s.seg_len = block_of, seg_len
    s.nch, s.oq, s.groups, s.NCHG, s.TOTCOL = nch, oq, groups, NCHG, TOTCOL
    s.icol, s.TOTICOL = icol, TOTICOL
    return s


def _core_tensors(s, c):
    """Per-core dstl / esrc16 / disT arrays following the shared schedule."""
    NB, NQ, P16 = s.NB, s.NQ, P // 16

    esrc16 = np.zeros((P, s.TOTICOL), np.int16)
    dstl = np.full((P, s.TOTCOL), PAD_LANE, np.float32)

    # per (position, q): this core's slot data
    seg_src = {}
    seg_lane = {}
    for p in range(NB):
        tb = s.block_of[c, p]
        for q in range(NQ):
            o = s.offs[tb * NQ + q]
            n = s.cnt[tb, q]
            seg_src[p, q] = s.k_src[o : o + n] - s.qbase[q]
            seg_lane[p, q] = s.k_lane[o : o + n]

    j = 0
    for g in range(s.NGRP):
        ps = list(range(g * GRP, (g + 1) * GRP))
        # idx lists per call
        for q in range(NQ):
            L = int(s.nch[g, q]) * P
            seq = np.zeros(L, np.int16)
            t = 0
            for p in ps:
                n = len(seg_src[p, q])
                seq[t : t + n] = seg_src[p, q].astype(np.int16)
                t += int(s.seg_len[p, q])
            w = seq.reshape(L // 16, 16).T  # idx i -> [i%16, i//16]
            io = s.icol[g, q]
            esrc16[:, io : io + L // 16] = np.tile(w, (8, 1))
        # dstl columns (emission order: block-major within group)
        for p, cols in s.groups[g]:
            for q, ci, t0, t1 in cols:
                lanes = seg_lane[p, q]
                n = len(lanes)
                lo = max(ci * P, t0)
                hi = min(ci * P + P, t0 + n)  # real (unpadded) slots only
                if hi > lo:
                    dstl[lo - ci * P : hi - ci * P, j] = lanes[lo - t0 : hi - t0]
                j += 1
    assert j == s.TOTCOL

    gids = s.block_of[c][:, None] * P + np.arange(P)[None, :]  # [NB, P]
    nodes = s.node_of_gid[gids]
    disT = np.ones((P, NB), np.float32)
    valid = nodes >= 0
    disT.T[valid] = s.dis[nodes[valid]]

    return (
        np.ascontiguousarray(esrc16),
        np.ascontiguousarray(dstl),
        np.ascontiguousarray(disT),
        nodes,
    )


# --------------------------------------------------------------------------
# Device program
# --------------------------------------------------------------------------


def _build_program(s, IN_CH, HID, OUT_CH, has_b1, has_b2):
    NB, NQ, NCHG, NGRP = s.NB, s.NQ, s.NCHG, s.NGRP
    KT = IN_CH // P
    assert IN_CH % P == 0 and HID == P and OUT_CH <= P

    nc = bass.Bass()
    xdis = nc.dram_tensor("xdis", [s.NTOT, IN_CH], BF16, kind="ExternalInput")
    W1 = nc.dram_tensor("W1", [IN_CH, HID], BF16, kind="ExternalInput")
    W2p = nc.dram_tensor("W2p", [HID, P], BF16, kind="ExternalInput")
    b1bc = nc.dram_tensor("b1bc", [P, HID], F32, kind="ExternalInput")
    b2bc = nc.dram_tensor("b2bc", [P, OUT_CH], F32, kind="ExternalInput")
    disT = nc.dram_tensor("disT", [P, NB], F32, kind="ExternalInput")
    esrc = nc.dram_tensor("esrc", [P, s.TOTICOL], I16, kind="ExternalInput")
    dstl = nc.dram_tensor("dstl", [P, s.TOTCOL], F32, kind="ExternalInput")
    outY = nc.dram_tensor("outY", [s.NPC, OUT_CH], F32, kind="ExternalOutput")

    h2s = nc.dram_tensor("h2s", [s.NPC, P], BF16)
    h2f = nc.dram_tensor("h2f", [s.NTOT, P], BF16)

    rg = [list(range(NCORES))]
    RELU = mybir.ActivationFunctionType.Relu
    ADD = mybir.AluOpType.add
    ISEQ = mybir.AluOpType.is_equal

    with tile.TileContext(nc) as tc:
        with tc.tile_pool(name="const", bufs=1) as cst:
            iotasb = cst.tile([P, P], BF16)
            nc.gpsimd.iota(
                iotasb[:],
                pattern=[[1, P]],
                base=0,
                channel_multiplier=0,
                allow_small_or_imprecise_dtypes=True,
            )
            idsb = cst.tile([P, P], BF16)
            make_identity(nc, idsb[:])
            # custom-op ucode (dma_gather) — after the stock gpsimd ops above
            nc.gpsimd.load_library(library_config.mlp)
            nidx_regs = {}
            for v in sorted({int(v) * P for v in np.unique(s.nch) if v}):
                nidx_regs[v] = nc.gpsimd.to_reg(v)

            w1sb = cst.tile([P, KT * HID], BF16)
            for k in range(KT):
                nc.sync.dma_start(
                    out=w1sb[:, k * HID : (k + 1) * HID],
                    in_=W1[k * P : (k + 1) * P, :],
                )
            w2sb = cst.tile([P, P], BF16)
            nc.sync.dma_start(out=w2sb[:], in_=W2p[:, :])
            dissb = cst.tile([P, NB], F32)
            nc.sync.dma_start(out=dissb[:], in_=disT[:, :])
            esrcsb = cst.tile([P, s.TOTICOL], I16)
            nc.sync.dma_start(out=esrcsb[:], in_=esrc[:, :])
            dstlsb = cst.tile([P, s.TOTCOL], F32)
            nc.sync.dma_start(out=dstlsb[:], in_=dstl[:, :])
            if has_b1:
                b1sb = cst.tile([P, HID], F32)
                nc.sync.dma_start(out=b1sb[:], in_=b1bc[:, :])
            if has_b2:
                b2sb = cst.tile([P, OUT_CH], F32)
                nc.sync.dma_start(out=b2sb[:], in_=b2bc[:, :])

            # ---------------- layer 1: aggregate xdis, emit h2 table -------
            with (
                tc.tile_pool(name="g1", bufs=2) as pg,
                tc.tile_pool(name="m1", bufs=6) as pm,
                tc.tile_pool(name="s1", bufs=3) as psb,
                tc.tile_pool(name="z1", bufs=2, space="PSUM") as pz,
                tc.tile_pool(name="t1", bufs=2, space="PSUM") as pt,
            ):
                j = 0
                for g in range(NGRP):
                    gt = pg.tile([P, NCHG, IN_CH], BF16, tag="gt")
                    for q in range(NQ):
                        nq = int(s.nch[g, q])
                        if nq == 0:
                            continue
                        o = int(s.oq[g, q])
                        io = int(s.icol[g, q])
                        nidx = nq * P
                        nc.gpsimd.dma_gather(
                            gt[:, o : o + nq, :],
                            xdis[s.qbase[q] : s.qbase[q] + s.qrows[q], :],
                            esrcsb[:, io : io + nq * (P // 16)],
                            nidx,
                            nidx_regs[nidx],
                            IN_CH,
                            single_packet=False,
                        )
                    for p, cols in s.groups[g]:
                        zt = pz.tile([P, IN_CH], F32, tag="z")
                        ncol = len(cols)
                        for ki, (q, ci, _t0, _t1) in enumerate(cols):
                            m = pm.tile([P, P], BF16, tag="m")
                            nc.vector.tensor_scalar(
                                m[:], iotasb[:], dstlsb[:, j : j + 1], None, ISEQ
                            )
                            j += 1
                            nc.tensor.matmul(
                                zt[:],
                                lhsT=m[:],
                                rhs=gt[:, int(s.oq[g, q]) + ci, :],
                                start=(ki == 0),
                                stop=(ki == ncol - 1),
                            )
                        # dense epilogue for block at position p
                        zsb = psb.tile([P, IN_CH], BF16, tag="zsb")
                        nc.scalar.copy(out=zsb[:], in_=zt[:])
                        zt_ts = psb.tile([P, KT * P], BF16, tag="zts")
                        for k in range(KT):
                            tr = pt.tile([P, P], BF16, tag="tr")
                            nc.tensor.transpose(
                                out=tr[:],
                                in_=zsb[:, k * P : (k + 1) * P],
                                identity=idsb[:],
                            )
                            nc.scalar.copy(
                                out=zt_ts[:, k * P : (k + 1) * P], in_=tr[:]
                            )
                        o1ps = pt.tile([P, HID], F32, tag="tp")
                        for k in range(KT):
                            nc.tensor.matmul(
                                o1ps[:],
                                lhsT=zt_ts[:, k * P : (k + 1) * P],
                                rhs=w1sb[:, k * HID : (k + 1) * HID],
                                start=(k == 0),
                                stop=(k == KT - 1),
                            )
                        o1sb = psb.tile([P, HID], BF16, tag="o1")
                        if has_b1:
                            t0f = psb.tile([P, HID], F32, tag="t0f")
                            nc.vector.tensor_scalar_mul(
                                t0f[:], o1ps[:], dissb[:, p : p + 1]
                            )
                            nc.vector.tensor_tensor(t0f[:], t0f[:], b1sb[:], op=ADD)
                            nc.scalar.activation(o1sb[:], t0f[:], RELU)
                        else:
                            nc.scalar.activation(
                                o1sb[:], o1ps[:], RELU, scale=dissb[:, p : p + 1]
                            )
                        trp = pt.tile([P, HID], BF16, tag="tr")
                        nc.tensor.transpose(out=trp[:], in_=o1sb[:], identity=idsb[:])
                        o1t = psb.tile([P, HID], BF16, tag="o1t")
                        nc.scalar.copy(out=o1t[:], in_=trp[:])
                        h2ps = pt.tile([P, P], F32, tag="tp")
                        nc.tensor.matmul(
                            h2ps[:], lhsT=o1t[:], rhs=w2sb[:], start=True, stop=True
                        )
                        h2sb = psb.tile([P, P], BF16, tag="h2")
                        nc.vector.tensor_scalar_mul(
                            h2sb[:], h2ps[:], dissb[:, p : p + 1]
                        )
                        nc.sync.dma_start(
                            out=h2s[p * P : (p + 1) * P, :], in_=h2sb[:]
                        )
                assert j == s.TOTCOL

            # ---------------- AllGather of the h2 message table ------------
            nc.gpsimd.collective_compute(
                "AllGather",
                mybir.AluOpType.bypass,
                replica_groups=rg,
                ins=[h2s[:, :]],
                outs=[h2f[:, :]],
            )

            # ---------------- layer 2: aggregate h2, write output ----------
            with (
                tc.tile_pool(name="g2", bufs=2) as pg2,
                tc.tile_pool(name="m2", bufs=6) as pm2,
                tc.tile_pool(name="s2", bufs=3) as psb2,
                tc.tile_pool(name="z2", bufs=2, space="PSUM") as pz2,
            ):
                j = 0
                for g in range(NGRP):
                    gt = pg2.tile([P, NCHG, P], BF16, tag="gt")
                    for q in range(NQ):
                        nq = int(s.nch[g, q])
                        if nq == 0:
                            continue
                        o = int(s.oq[g, q])
                        io = int(s.icol[g, q])
                        nidx = nq * P
                        nc.gpsimd.dma_gather(
                            gt[:, o : o + nq, :],
                            h2f[s.qbase[q] : s.qbase[q] + s.qrows[q], :],
                            esrcsb[:, io : io + nq * (P // 16)],
                            nidx,
                            nidx_regs[nidx],
                            P,
                            single_packet=False,
                        )
                    for p, cols in s.groups[g]:
                        zt = pz2.tile([P, P], F32, tag="z")
                        ncol = len(cols)
                        for ki, (q, ci, _t0, _t1) in enumerate(cols):
                            m = pm2.tile([P, P], BF16, tag="m")
                            nc.vector.tensor_scalar(
                                m[:], iotasb[:], dstlsb[:, j : j + 1], None, ISEQ
                            )
                            j += 1
                            nc.tensor.matmul(
                                zt[:],
                                lhsT=m[:],
                                rhs=gt[:, int(s.oq[g, q]) + ci, :],
                                start=(ki == 0),
                                stop=(ki == ncol - 1),
                            )
                        osb = psb2.tile([P, OUT_CH], F32, tag="o")
                        if has_b2:
                            t2f = psb2.tile([P, OUT_CH], F32, tag="t2f")
                            nc.vector.tensor_scalar_mul(
                                t2f[:], zt[:, :OUT_CH], dissb[:, p : p + 1]
                            )
                            nc.vector.tensor_tensor(t2f[:], t2f[:], b2sb[:], op=ADD)
                            nc.scalar.activation(osb[:], t2f[:], RELU)
                        else:
                            nc.scalar.activation(
                                osb[:],
                                zt[:, :OUT_CH],
                                RELU,
                                scale=dissb[:, p : p + 1],
                            )
                        nc.sync.dma_start(
                            out=outY[p * P : (p + 1) * P, :], in_=osb[:]
                        )
                assert j == s.TOTCOL

    _spill_waits(nc)
    mybir.codegen_inst_isa_subclasses(nc)
    return nc


# --------------------------------------------------------------------------
# Entry point
# --------------------------------------------------------------------------


def kernel(x, edge_index, W1, b1, W2, b2):
    x = np.asarray(x, dtype=np.float32)
    W1 = np.asarray(W1, dtype=np.float32)
    W2 = np.asarray(W2, dtype=np.float32)
    b1 = np.asarray(b1, dtype=np.float32)
    b2 = np.asarray(b2, dtype=np.float32)
    src = np.asarray(edge_index[0]).astype(np.int64)
    dst = np.asarray(edge_index[1]).astype(np.int64)

    N, IN_CH = x.shape
    HID = W1.shape[1]
    OUT_CH = W2.shape[1]

    s = _prepare(N, src, dst)

    # full xdis replica, gid-indexed (empty gids = 0)
    xdis = np.zeros((s.NTOT, IN_CH), ml_dtypes.bfloat16)
    xdis[s.gid_of] = (x * s.dis[:, None]).astype(ml_dtypes.bfloat16)

    W1b = np.ascontiguousarray(W1.astype(ml_dtypes.bfloat16))
    W2p = np.zeros((HID, P), ml_dtypes.bfloat16)
    W2p[:, :OUT_CH] = W2.astype(ml_dtypes.bfloat16)
    has_b1 = bool(np.any(b1))
    has_b2 = bool(np.any(b2))
    b1bc = np.ascontiguousarray(np.broadcast_to(b1, (P, HID)).astype(np.float32))
    b2bc = np.ascontiguousarray(np.broadcast_to(b2, (P, OUT_CH)).astype(np.float32))

    nc = _build_program(s, IN_CH, HID, OUT_CH, has_b1, has_b2)

    in_maps = []
    node_maps = []
    for c in range(NCORES):
        esrc16, dstlc, disTc, nodes = _core_tensors(s, c)
        node_maps.append(nodes)
        in_maps.append(
            {
                "xdis": xdis,
                "W1": W1b,
                "W2p": W2p,
                "b1bc": b1bc,
                "b2bc": b2bc,
                "disT": disTc,
                "esrc": esrc16,
                "dstl": dstlc,
            }
        )

    res = run_bass_kernel_spmd(nc, in_maps, core_ids=list(range(NCORES)))
    global _last_results, _last_nc
    _last_results = res
    _last_nc = nc

    out = np.empty((N, OUT_CH), np.float32)
    for c in range(NCORES):
        oc = res.results[c]["outY"]  # [NPC, OUT_CH], row = p*128+lane
        nodes = node_maps[c].reshape(-1)  # [NB*P] original node or -1
        valid = nodes >= 0
        out[nodes[valid]] = oc[valid]
    return out


# revision 9
# speedup vs baseline: 3.0874x; 1.0195x over previous
"""2-layer GCN (PyG GCNConv x2 + ReLU) on 8 Trainium2 NeuronCores.

Math per layer (A from edge_index, deg = indeg(dst)+1, dis = deg^-1/2):
    out[d] = relu( dis_d * ( sum_{e: s->d} dis_s*h[s] + dis_d*h[d] ) @ W + b )

Strategy (v2 — replaces per-chunk indirect DMAs of the first version):
  * Layer 1 is computed aggregate-then-transform: every core holds a full
    replica of xdis = x*dis (bf16) in its HBM, so there is NO layer-1
    collective at all.  Each core aggregates Z = sum xdis[src] for its own
    destination blocks with large dma_gather calls (custom SWDGE gather,
    ~1us fixed cost amortized over thousands of rows) + one-hot-mask
    matmuls, then applies W1 (and W2, producing the layer-2 message table
    h2 = dis*(relu(.)@W2)) densely per 128-node block.
  * One AllGather shares the h2 tables (padded to 128 cols so gather rows
    are 256B), then layer 2 aggregates the same way.
  * Self-loops are folded in as ordinary edges (src == dst).
  * dma_gather uses int16 indices, so the node table is split in 4 row
    slices of <=32768 and edge slots are grouped by src-slice.  The slot
    schedule (segment sizes, chunk/block spans) is made identical across
    cores by padding each (block, slice) segment to the max over cores,
    keeping the SPMD program uniform; only tensor data differs per core.
"""

import math

import ml_dtypes
import numpy as np

import concourse.bass as bass
import concourse.mybir as mybir
import concourse.tile as tile
from concourse import library_config
from concourse.bass_utils import run_bass_kernel_spmd
from concourse.masks import make_identity
from concourse.vector_clock import ScopedClock

P = 128
NCORES = 8
PAD_LANE = 1000.0  # dst-lane sentinel for padding edge slots (one-hot = 0)
QS = 32768  # dma_gather int16 index range -> table row-slice size
GRP = 7  # dst blocks per gather group
KB = 8  # one-hot mask columns built per DVE instruction

F32 = mybir.dt.float32
BF16 = mybir.dt.bfloat16
I16 = mybir.dt.int16


def _patched_drain_and_barrier(self, tick_clock, wait_clock):
    # This walrus build rejects >1 sem wait on TPB_CTRL (Drain) instructions.
    # Spill the tile-epilogue drain waits onto extra single-wait drains.
    drain_inst = self.nc.sync.drain()
    wait_clock.add_sem_waits(
        drain_inst.ins, ScopedClock({None: tick_clock.global_clock})
    )
    si = drain_inst.ins.sync_info
    waits = list(si.on_wait)
    if len(waits) > 1:
        while len(si.on_wait):
            si.on_wait.pop()
        si.on_wait.append(waits[0])
        for w in waits[1:]:
            d2 = self.nc.sync.drain(fusable=False)
            si2 = d2.ins.sync_info
            if si2 is None:
                d2.ins.sync_info = mybir.SyncInfo(on_wait=[w], on_update=[])
            else:
                si2.on_wait.append(w)
    self.nc.all_engine_barrier()
    popped = self.nc._tile_sem_poison_stack.pop()
    assert popped is self._sem_poison
    self.nc.clear_and_free_semaphores(list(self.sems.allocated().values()))
    self.nc.all_engine_barrier()


tile.TileContext._drain_and_barrier = _patched_drain_and_barrier


def _spill_waits(nc, max_waits=1):
    """This walrus build accepts at most one sync wait per instruction.
    Move extra waits onto dedicated single-wait NoOps ahead of the
    instruction on the same engine (engines execute in program order)."""
    n = 0
    for f in nc.m.functions:
        for blk in f.blocks:
            il = blk.instructions
            out = []
            for inst in il:
                si = inst.sync_info
                if si is not None and len(si.on_wait) > max_waits:
                    waits = list(si.on_wait)
                    while len(si.on_wait):
                        si.on_wait.pop()
                    for w in waits[:max_waits]:
                        si.on_wait.append(w)
                    for w in waits[max_waits:]:
                        nop = mybir.InstNoOp(
                            name=f"waitspill-{n}",
                            sync_info=mybir.SyncInfo(on_wait=[w], on_update=[]),
                            bass_nofuse=True,
                            engine=inst.engine,
                        )
                        n += 1
                        out.append(nop)
                out.append(inst)
            blk.instructions = out
    return n


# --------------------------------------------------------------------------
# Host-side schedule construction
# --------------------------------------------------------------------------


class _Sched:
    pass


def _prepare(N, src, dst):
    """Node->gid assignment, slot buckets, and the (core-uniform) gather /
    column schedule."""
    s = _Sched()
    NB = int(math.ceil(N / (NCORES * P)))
    TB = NB * NCORES
    NPC = NB * P
    NTOT = TB * P
    NGRP = int(math.ceil(NB / GRP))
    assert NGRP * GRP == NB, (NB, NGRP)
    NQ = int(math.ceil(NTOT / QS))
    qbase = [q * QS for q in range(NQ)]
    qrows = [min(QS, NTOT - q * QS) for q in range(NQ)]

    E = src.shape[0]
    indeg = np.bincount(dst, minlength=N).astype(np.int64)
    dis = (1.0 / np.sqrt(indeg.astype(np.float64) + 1.0)).astype(np.float32)

    # snake assignment of degree-sorted nodes over TB blocks
    order = np.argsort(-indeg, kind="stable")
    i = np.arange(N)
    rnd = i // TB
    pos = i % TB
    blk_i = np.where(rnd % 2 == 0, pos, TB - 1 - pos)
    assert rnd.max() < P
    gid_of = np.empty(N, np.int64)
    gid_of[order] = blk_i * P + rnd
    node_of_gid = np.full(NTOT, -1, np.int64)
    node_of_gid[gid_of] = np.arange(N)

    # slots = edges + self-loops, bucketed by (dst block, src slice)
    all_src = np.concatenate([gid_of[src], gid_of])
    all_dst = np.concatenate([gid_of[dst], gid_of])
    sblk = all_dst >> 7
    slane = (all_dst & 127).astype(np.float32)
    sq = all_src // QS
    key = sblk * NQ + sq
    ord2 = np.argsort(key, kind="stable")
    k_src = all_src[ord2]
    k_lane = slane[ord2]
    cnt = np.bincount(key, minlength=TB * NQ).reshape(TB, NQ)
    offs = np.zeros(TB * NQ + 1, np.int64)
    np.cumsum(cnt.reshape(-1), out=offs[1:])

    # global block -> (core, position): sorted by total slots, snake
    btot = cnt.sum(1)
    bord = np.argsort(-btot, kind="stable")
    block_of = np.empty((NCORES, NB), np.int64)
    for p in range(NB):
        row = bord[p * NCORES : (p + 1) * NCORES]
        if p % 2:
            row = row[::-1]
        block_of[:, p] = row

    # renumber into PHYSICAL gid space: pgid = (core*NB + position)*128 + lane.
    # The AllGathered h2f table is laid out in physical order, so gather
    # indices, slice bucketing, the xdis table, and outputs all use pgid.
    phys_pos = np.empty(TB, np.int64)
    for c in range(NCORES):
        for p in range(NB):
            phys_pos[block_of[c, p]] = c * NB + p
    pgid_of = phys_pos[gid_of >> 7] * P + (gid_of & 127)
    gid_of = pgid_of
    node_of_gid = np.full(NTOT, -1, np.int64)
    node_of_gid[gid_of] = np.arange(N)

    # redo slot bucketing in physical space
    all_src = np.concatenate([gid_of[src], gid_of])
    all_dst = np.concatenate([gid_of[dst], gid_of])
    sblk = all_dst >> 7
    slane = (all_dst & 127).astype(np.float32)
    sq = all_src // QS
    key = sblk * NQ + sq
    ord2 = np.argsort(key, kind="stable")
    k_src = all_src[ord2]
    k_lane = slane[ord2]
    cnt = np.bincount(key, minlength=TB * NQ).reshape(TB, NQ)
    offs = np.zeros(TB * NQ + 1, np.int64)
    np.cumsum(cnt.reshape(-1), out=offs[1:])
    block_of = (np.arange(NCORES)[:, None] * NB) + np.arange(NB)[None, :]

    # uniform per-(position, slice) segment lengths
    seg_len = cnt[block_of, :].max(axis=0)  # [NB, NQ]

    # per-group call sizes + column schedule (shared by both layers)
    nch = np.zeros((NGRP, NQ), np.int64)
    oq = np.zeros((NGRP, NQ), np.int64)  # chunk offset of call q in group tile
    groups = []  # per group: list of (p, [(q, ci, t0, t1, start, stop)])
    for g in range(NGRP):
        ps = list(range(g * GRP, (g + 1) * GRP))
        for q in range(NQ):
            L = int(seg_len[ps, q].sum())
            nch[g, q] = (L + P - 1) // P
        oq[g] = np.concatenate([[0], np.cumsum(nch[g])[:-1]])
        blocks = []
        for bi, p in enumerate(ps):
            cols = []
            for q in range(NQ):
                t0 = int(seg_len[ps[:bi], q].sum())
                t1 = t0 + int(seg_len[p, q])
                if t1 == t0:
                    continue
                for ci in range(t0 // P, (t1 - 1) // P + 1):
                    cols.append((q, ci, t0, t1))
            assert cols
            blocks.append((p, cols))
        groups.append(blocks)
    NCHG = int(nch.sum(1).max())
    TOTCOL = sum(len(c) for blks in groups for _, c in blks)

    # int16 idx column offsets per call
    icol = np.zeros((NGRP, NQ), np.int64)
    run = 0
    for g in range(NGRP):
        for q in range(NQ):
            icol[g, q] = run
            run += int(nch[g, q]) * (P // 16)
    TOTICOL = int(run)

    s.NB, s.TB, s.NPC, s.NTOT, s.NGRP, s.NQ = NB, TB, NPC, NTOT, NGRP, NQ
    s.qbase, s.qrows = qbase, qrows
    s.dis, s.gid_of, s.node_of_gid = dis, gid_of, node_of_gid
    s.k_src, s.k_lane, s.cnt, s.offs = k_src, k_lane, cnt, offs
    s.block_of, s.seg_len = block_of, seg_len
    s.nch, s.oq, s.groups, s.NCHG, s.TOTCOL = nch, oq, groups, NCHG, TOTCOL
    s.icol, s.TOTICOL = icol, TOTICOL
    return s


def _core_tensors(s, c):
    """Per-core dstl / esrc16 / disT arrays following the shared schedule."""
    NB, NQ, P16 = s.NB, s.NQ, P // 16

    esrc16 = np.zeros((P, s.TOTICOL), np.int16)
    dstl = np.full((P, s.TOTCOL + KB), PAD_LANE, np.float32)

    # per (position, q): this core's slot data
    seg_src = {}
    seg_lane = {}
    for p in range(NB):
        tb = s.block_of[c, p]
        for q in range(NQ):
            o = s.offs[tb * NQ + q]
            n = s.cnt[tb, q]
            seg_src[p, q] = s.k_src[o : o + n] - s.qbase[q]
            seg_lane[p, q] = s.k_lane[o : o + n]

    j = 0
    for g in range(s.NGRP):
        ps = list(range(g * GRP, (g + 1) * GRP))
        # idx lists per call
        for q in range(NQ):
            L = int(s.nch[g, q]) * P
            seq = np.zeros(L, np.int16)
            t = 0
            for p in ps:
                n = len(seg_src[p, q])
                seq[t : t + n] = seg_src[p, q].astype(np.int16)
                t += int(s.seg_len[p, q])
            w = seq.reshape(L // 16, 16).T  # idx i -> [i%16, i//16]
            io = s.icol[g, q]
            esrc16[:, io : io + L // 16] = np.tile(w, (8, 1))
        # dstl columns (emission order: block-major within group)
        for p, cols in s.groups[g]:
            for q, ci, t0, t1 in cols:
                lanes = seg_lane[p, q]
                n = len(lanes)
                lo = max(ci * P, t0)
                hi = min(ci * P + P, t0 + n)  # real (unpadded) slots only
                if hi > lo:
                    dstl[lo - ci * P : hi - ci * P, j] = lanes[lo - t0 : hi - t0]
                j += 1
    assert j == s.TOTCOL

    gids = s.block_of[c][:, None] * P + np.arange(P)[None, :]  # [NB, P]
    nodes = s.node_of_gid[gids]
    disT = np.ones((P, NB), np.float32)
    valid = nodes >= 0
    disT.T[valid] = s.dis[nodes[valid]]

    return (
        np.ascontiguousarray(esrc16),
        np.ascontiguousarray(dstl.astype(ml_dtypes.bfloat16)),
        np.ascontiguousarray(disT),
        nodes,
    )


# --------------------------------------------------------------------------
# Device program
# --------------------------------------------------------------------------


def _build_program(s, IN_CH, HID, OUT_CH, has_b1, has_b2):
    NB, NQ, NCHG, NGRP = s.NB, s.NQ, s.NCHG, s.NGRP
    KT = IN_CH // P
    assert IN_CH % P == 0 and HID == P and OUT_CH <= P

    nc = bass.Bass()
    xdis = nc.dram_tensor("xdis", [s.NTOT, IN_CH], BF16, kind="ExternalInput")
    W1 = nc.dram_tensor("W1", [IN_CH, HID], BF16, kind="ExternalInput")
    W2p = nc.dram_tensor("W2p", [HID, P], BF16, kind="ExternalInput")
    b1bc = nc.dram_tensor("b1bc", [P, HID], F32, kind="ExternalInput")
    b2bc = nc.dram_tensor("b2bc", [P, OUT_CH], F32, kind="ExternalInput")
    disT = nc.dram_tensor("disT", [P, NB], F32, kind="ExternalInput")
    iotaK = nc.dram_tensor("iotaK", [P, P * KB], BF16, kind="ExternalInput")
    esrc = nc.dram_tensor("esrc", [P, s.TOTICOL], I16, kind="ExternalInput")
    dstl = nc.dram_tensor("dstl", [P, s.TOTCOL + KB], BF16, kind="ExternalInput")
    outY = nc.dram_tensor("outY", [s.NPC, OUT_CH], F32, kind="ExternalOutput")

    h2s = nc.dram_tensor("h2s", [s.NPC, P], BF16)
    h2f = nc.dram_tensor("h2f", [s.NTOT, P], BF16)

    rg = [list(range(NCORES))]
    RELU = mybir.ActivationFunctionType.Relu
    ADD = mybir.AluOpType.add
    ISEQ = mybir.AluOpType.is_equal

    with tile.TileContext(nc) as tc:
        with tc.tile_pool(name="const", bufs=1) as cst:
            iotaKsb = cst.tile([P, P * KB], BF16)
            nc.sync.dma_start(out=iotaKsb[:], in_=iotaK[:, :])
            idsb = cst.tile([P, P], BF16)
            make_identity(nc, idsb[:])
            # custom-op ucode (dma_gather) — after the stock gpsimd ops above
            nc.gpsimd.load_library(library_config.mlp)
            nidx_regs = {}
            for v in sorted({int(v) * P for v in np.unique(s.nch) if v}):
                nidx_regs[v] = nc.gpsimd.to_reg(v)

            w1sb = cst.tile([P, KT * HID], BF16)
            for k in range(KT):
                nc.sync.dma_start(
                    out=w1sb[:, k * HID : (k + 1) * HID],
                    in_=W1[k * P : (k + 1) * P, :],
                )
            w2sb = cst.tile([P, P], BF16)
            nc.sync.dma_start(out=w2sb[:], in_=W2p[:, :])
            dissb = cst.tile([P, NB], F32)
            nc.sync.dma_start(out=dissb[:], in_=disT[:, :])
            esrcsb = cst.tile([P, s.TOTICOL], I16)
            nc.sync.dma_start(out=esrcsb[:], in_=esrc[:, :])
            dstlsb = cst.tile([P, s.TOTCOL + KB], BF16)
            nc.sync.dma_start(out=dstlsb[:], in_=dstl[:, :])
            if has_b1:
                b1sb = cst.tile([P, HID], F32)
                nc.sync.dma_start(out=b1sb[:], in_=b1bc[:, :])
            if has_b2:
                b2sb = cst.tile([P, OUT_CH], F32)
                nc.sync.dma_start(out=b2sb[:], in_=b2bc[:, :])

            # ---------------- layer 1: aggregate xdis, emit h2 table -------
            with (
                tc.tile_pool(name="g1", bufs=2) as pg,
                tc.tile_pool(name="m1", bufs=6) as pm,
                tc.tile_pool(name="s1", bufs=3) as psb,
                tc.tile_pool(name="z1", bufs=2, space="PSUM") as pz,
                tc.tile_pool(name="t1", bufs=2, space="PSUM") as pt,
            ):
                j = 0
                for g in range(NGRP):
                    gt = pg.tile([P, NCHG, IN_CH], BF16, tag="gt")
                    for q in range(NQ):
                        nq = int(s.nch[g, q])
                        if nq == 0:
                            continue
                        o = int(s.oq[g, q])
                        io = int(s.icol[g, q])
                        nidx = nq * P
                        nc.gpsimd.dma_gather(
                            gt[:, o : o + nq, :],
                            xdis[s.qbase[q] : s.qbase[q] + s.qrows[q], :],
                            esrcsb[:, io : io + nq * (P // 16)],
                            nidx,
                            nidx_regs[nidx],
                            IN_CH,
                            single_packet=False,
                        )
                    j0c = j
                    cig = 0
                    cur = None
                    for p, cols in s.groups[g]:
                        zt = pz.tile([P, IN_CH], F32, tag="z")
                        ncol = len(cols)
                        for ki, (q, ci, _t0, _t1) in enumerate(cols):
                            bb, r = divmod(cig, KB)
                            if r == 0:
                                cur = pm.tile([P, P * KB], BF16, tag="m")
                                nc.vector.tensor_tensor(
                                    cur[:].rearrange("p (l k) -> p l k", k=KB),
                                    iotaKsb[:].rearrange("p (l k) -> p l k", k=KB),
                                    dstlsb[:, j0c + bb * KB : j0c + (bb + 1) * KB]
                                    .unsqueeze(1)
                                    .broadcast_to([P, P, KB]),
                                    op=ISEQ,
                                )
                            cig += 1
                            j += 1
                            mv = cur[:].rearrange("p (l k) -> p k l", k=KB)
                            nc.tensor.matmul(
                                zt[:],
                                lhsT=mv[:, r, :],
                                rhs=gt[:, int(s.oq[g, q]) + ci, :],
                                start=(ki == 0),
                                stop=(ki == ncol - 1),
                            )
                        # dense epilogue for block at position p
                        zsb = psb.tile([P, IN_CH], BF16, tag="zsb")
                        nc.scalar.copy(out=zsb[:], in_=zt[:])
                        zt_ts = psb.tile([P, KT * P], BF16, tag="zts")
                        for k in range(KT):
                            tr = pt.tile([P, P], BF16, tag="tr")
                            nc.tensor.transpose(
                                out=tr[:],
                                in_=zsb[:, k * P : (k + 1) * P],
                                identity=idsb[:],
                            )
                            nc.scalar.copy(
                                out=zt_ts[:, k * P : (k + 1) * P], in_=tr[:]
                            )
                        o1ps = pt.tile([P, HID], F32, tag="tp")
                        for k in range(KT):
                            nc.tensor.matmul(
                                o1ps[:],
                                lhsT=zt_ts[:, k * P : (k + 1) * P],
                                rhs=w1sb[:, k * HID : (k + 1) * HID],
                                start=(k == 0),
                                stop=(k == KT - 1),
                            )
                        o1sb = psb.tile([P, HID], BF16, tag="o1")
                        if has_b1:
                            t0f = psb.tile([P, HID], F32, tag="t0f")
                            nc.vector.tensor_scalar_mul(
                                t0f[:], o1ps[:], dissb[:, p : p + 1]
                            )
                            nc.vector.tensor_tensor(t0f[:], t0f[:], b1sb[:], op=ADD)
                            nc.scalar.activation(o1sb[:], t0f[:], RELU)
                        else:
                            nc.scalar.activation(
                                o1sb[:], o1ps[:], RELU, scale=dissb[:, p : p + 1]
                            )
                        trp = pt.tile([P, HID], BF16, tag="tr")
                        nc.tensor.transpose(out=trp[:], in_=o1sb[:], identity=idsb[:])
                        o1t = psb.tile([P, HID], BF16, tag="o1t")
                        nc.scalar.copy(out=o1t[:], in_=trp[:])
                        h2ps = pt.tile([P, P], F32, tag="tp")
                        nc.tensor.matmul(
                            h2ps[:], lhsT=o1t[:], rhs=w2sb[:], start=True, stop=True
                        )
                        h2sb = psb.tile([P, P], BF16, tag="h2")
                        nc.vector.tensor_scalar_mul(
                            h2sb[:], h2ps[:], dissb[:, p : p + 1]
                        )
                        nc.sync.dma_start(
                            out=h2s[p * P : (p + 1) * P, :], in_=h2sb[:]
                        )
                assert j == s.TOTCOL

            # ---------------- AllGather of the h2 message table ------------
            nc.gpsimd.collective_compute(
                "AllGather",
                mybir.AluOpType.bypass,
                replica_groups=rg,
                ins=[h2s[:, :]],
                outs=[h2f[:, :]],
            )

            # ---------------- layer 2: aggregate h2, write output ----------
            with (
                tc.tile_pool(name="g2", bufs=2) as pg2,
                tc.tile_pool(name="m2", bufs=6) as pm2,
                tc.tile_pool(name="s2", bufs=3) as psb2,
                tc.tile_pool(name="z2", bufs=2, space="PSUM") as pz2,
            ):
                j = 0
                for g in range(NGRP):
                    gt = pg2.tile([P, NCHG, P], BF16, tag="gt")
                    for q in range(NQ):
                        nq = int(s.nch[g, q])
                        if nq == 0:
                            continue
                        o = int(s.oq[g, q])
                        io = int(s.icol[g, q])
                        nidx = nq * P
                        nc.gpsimd.dma_gather(
                            gt[:, o : o + nq, :],
                            h2f[s.qbase[q] : s.qbase[q] + s.qrows[q], :],
                            esrcsb[:, io : io + nq * (P // 16)],
                            nidx,
                            nidx_regs[nidx],
                            P,
                            single_packet=False,
                        )
                    j0c = j
                    cig = 0
                    cur = None
                    for p, cols in s.groups[g]:
                        zt = pz2.tile([P, P], F32, tag="z")
                        ncol = len(cols)
                        for ki, (q, ci, _t0, _t1) in enumerate(cols):
                            bb, r = divmod(cig, KB)
                            if r == 0:
                                cur = pm2.tile([P, P * KB], BF16, tag="m")
                                nc.vector.tensor_tensor(
                                    cur[:].rearrange("p (l k) -> p l k", k=KB),
                                    iotaKsb[:].rearrange("p (l k) -> p l k", k=KB),
                                    dstlsb[:, j0c + bb * KB : j0c + (bb + 1) * KB]
                                    .unsqueeze(1)
                                    .broadcast_to([P, P, KB]),
                                    op=ISEQ,
                                )
                            cig += 1
                            j += 1
                            mv = cur[:].rearrange("p (l k) -> p k l", k=KB)
                            nc.tensor.matmul(
                                zt[:],
                                lhsT=mv[:, r, :],
                                rhs=gt[:, int(s.oq[g, q]) + ci, :],
                                start=(ki == 0),
                                stop=(ki == ncol - 1),
                            )
                        osb = psb2.tile([P, OUT_CH], F32, tag="o")
                        if has_b2:
                            t2f = psb2.tile([P, OUT_CH], F32, tag="t2f")
                            nc.vector.tensor_scalar_mul(
                                t2f[:], zt[:, :OUT_CH], dissb[:, p : p + 1]
                            )
                            nc.vector.tensor_tensor(t2f[:], t2f[:], b2sb[:], op=ADD)
                            nc.scalar.activation(osb[:], t2f[:], RELU)
                        else:
                            nc.scalar.activation(
                                osb[:],
                                zt[:, :OUT_CH],
                                RELU,
                                scale=dissb[:, p : p + 1],
                            )
                        nc.sync.dma_start(
                            out=outY[p * P : (p + 1) * P, :], in_=osb[:]
                        )
                assert j == s.TOTCOL

    _spill_waits(nc)
    mybir.codegen_inst_isa_subclasses(nc)
    return nc


# --------------------------------------------------------------------------
# Entry point
# --------------------------------------------------------------------------


def kernel(x, edge_index, W1, b1, W2, b2):
    x = np.asarray(x, dtype=np.float32)
    W1 = np.asarray(W1, dtype=np.float32)
    W2 = np.asarray(W2, dtype=np.float32)
    b1 = np.asarray(b1, dtype=np.float32)
    b2 = np.asarray(b2, dtype=np.float32)
    src = np.asarray(edge_index[0]).astype(np.int64)
    dst = np.asarray(edge_index[1]).astype(np.int64)

    N, IN_CH = x.shape
    HID = W1.shape[1]
    OUT_CH = W2.shape[1]

    s = _prepare(N, src, dst)

    # full xdis replica, gid-indexed (empty gids = 0)
    xdis = np.zeros((s.NTOT, IN_CH), ml_dtypes.bfloat16)
    xdis[s.gid_of] = (x * s.dis[:, None]).astype(ml_dtypes.bfloat16)

    W1b = np.ascontiguousarray(W1.astype(ml_dtypes.bfloat16))
    W2p = np.zeros((HID, P), ml_dtypes.bfloat16)
    W2p[:, :OUT_CH] = W2.astype(ml_dtypes.bfloat16)
    has_b1 = bool(np.any(b1))
    has_b2 = bool(np.any(b2))
    b1bc = np.ascontiguousarray(np.broadcast_to(b1, (P, HID)).astype(np.float32))
    b2bc = np.ascontiguousarray(np.broadcast_to(b2, (P, OUT_CH)).astype(np.float32))

    nc = _build_program(s, IN_CH, HID, OUT_CH, has_b1, has_b2)

    iotaK = np.zeros((P, P * KB), ml_dtypes.bfloat16)
    for l in range(P):
        iotaK[:, l * KB : (l + 1) * KB] = l

    in_maps = []
    node_maps = []
    for c in range(NCORES):
        esrc16, dstlc, disTc, nodes = _core_tensors(s, c)
        node_maps.append(nodes)
        in_maps.append(
            {
                "xdis": xdis,
                "W1": W1b,
                "W2p": W2p,
                "b1bc": b1bc,
                "b2bc": b2bc,
                "disT": disTc,
                "iotaK": iotaK,
                "esrc": esrc16,
                "dstl": dstlc,
            }
        )

    res = run_bass_kernel_spmd(nc, in_maps, core_ids=list(range(NCORES)))
    global _last_results, _last_nc
    _last_results = res
    _last_nc = nc

    out = np.empty((N, OUT_CH), np.float32)
    for c in range(NCORES):
        oc = res.results[c]["outY"]  # [NPC, OUT_CH], row = p*128+lane
        nodes = node_maps[c].reshape(-1)  # [NB*P] original node or -1
        valid = nodes >= 0
        out[nodes[valid]] = oc[valid]
    return out


# revision 12
# speedup vs baseline: 3.1794x; 1.0298x over previous
"""2-layer GCN (PyG GCNConv x2 + ReLU) on 8 Trainium2 NeuronCores.

Math per layer (A from edge_index, deg = indeg(dst)+1, dis = deg^-1/2):
    out[d] = relu( dis_d * ( sum_{e: s->d} dis_s*h[s] + dis_d*h[d] ) @ W + b )

Strategy:
  * Layer 1 is aggregate-then-transform: every core holds a full replica of
    xdis = x*dis (bf16) in its HBM (NO layer-1 collective).  Each core
    aggregates Z = sum xdis[src] for its own destination blocks with large
    dma_gather calls + one-hot-mask matmuls, then applies W1 and W2 densely
    per 128-node block, producing the layer-2 message table h2 = dis*(relu@W2).
  * One AllGather shares the compact [NTOT, 64] bf16 h2 table.
  * Layer 2 gathers PAIRS of h2 rows (256B descriptors cover two nodes ->
    half the per-edge DMA cost); slots are parity-sorted inside each
    (block, slice) segment so each mask column selects one 64-col half.
  * Self-loops are folded in as ordinary edges (src == dst).
  * dma_gather uses int16 indices: the node table is split in 4 row slices
    of <=32768 (layer 1) / 2 pair-slices (layer 2).  The slot schedule is
    made identical across cores by padding each (block, slice, parity)
    sub-segment to the max over cores; only tensor data differs per core.
  * One-hot masks are built 8 columns per DVE instruction (interleaved iota
    constant + broadcast-AP tensor_tensor is_equal); the PE reads the mask
    slices with a strided lhsT access pattern.
"""

import math

import ml_dtypes
import numpy as np

import concourse.bass as bass
import concourse.mybir as mybir
import concourse.tile as tile
from concourse import library_config
from concourse.bass_utils import run_bass_kernel_spmd
from concourse.masks import make_identity
from concourse.vector_clock import ScopedClock

P = 128
NCORES = 8
PAD_LANE = 1000.0  # dst-lane sentinel for padding edge slots (one-hot = 0)
QS = 32768  # dma_gather int16 index range -> table row-slice size
GRP = 7  # dst blocks per gather group
KB = 8  # one-hot mask columns built per DVE instruction

F32 = mybir.dt.float32
BF16 = mybir.dt.bfloat16
I16 = mybir.dt.int16


def _patched_drain_and_barrier(self, tick_clock, wait_clock):
    # This walrus build rejects >1 sem wait on TPB_CTRL (Drain) instructions.
    # Spill the tile-epilogue drain waits onto extra single-wait drains.
    drain_inst = self.nc.sync.drain()
    wait_clock.add_sem_waits(
        drain_inst.ins, ScopedClock({None: tick_clock.global_clock})
    )
    si = drain_inst.ins.sync_info
    waits = list(si.on_wait)
    if len(waits) > 1:
        while len(si.on_wait):
            si.on_wait.pop()
        si.on_wait.append(waits[0])
        for w in waits[1:]:
            d2 = self.nc.sync.drain(fusable=False)
            si2 = d2.ins.sync_info
            if si2 is None:
                d2.ins.sync_info = mybir.SyncInfo(on_wait=[w], on_update=[])
            else:
                si2.on_wait.append(w)
    self.nc.all_engine_barrier()
    popped = self.nc._tile_sem_poison_stack.pop()
    assert popped is self._sem_poison
    self.nc.clear_and_free_semaphores(list(self.sems.allocated().values()))
    self.nc.all_engine_barrier()


tile.TileContext._drain_and_barrier = _patched_drain_and_barrier


def _spill_waits(nc, max_waits=1):
    """This walrus build accepts at most one sync wait per instruction.
    Move extra waits onto dedicated single-wait NoOps ahead of the
    instruction on the same engine (engines execute in program order)."""
    n = 0
    for f in nc.m.functions:
        for blk in f.blocks:
            il = blk.instructions
            out = []
            for inst in il:
                si = inst.sync_info
                if si is not None and len(si.on_wait) > max_waits:
                    waits = list(si.on_wait)
                    while len(si.on_wait):
                        si.on_wait.pop()
                    for w in waits[:max_waits]:
                        si.on_wait.append(w)
                    for w in waits[max_waits:]:
                        nop = mybir.InstNoOp(
                            name=f"waitspill-{n}",
                            sync_info=mybir.SyncInfo(on_wait=[w], on_update=[]),
                            bass_nofuse=True,
                            engine=inst.engine,
                        )
                        n += 1
                        out.append(nop)
                out.append(inst)
            blk.instructions = out
    return n


# --------------------------------------------------------------------------
# Host-side schedule construction
# --------------------------------------------------------------------------


class _Sched:
    pass


def _prepare(N, src, dst):
    """Node->gid assignment, slot buckets, and the (core-uniform) gather /
    column schedule for both layers."""
    s = _Sched()
    NB = int(math.ceil(N / (NCORES * P)))
    TB = NB * NCORES
    NPC = NB * P
    NTOT = TB * P
    NGRP = int(math.ceil(NB / GRP))
    assert NGRP * GRP == NB, (NB, NGRP)
    NQ = int(math.ceil(NTOT / QS))
    assert NQ % 2 == 0
    qbase = [q * QS for q in range(NQ)]
    qrows = [min(QS, NTOT - q * QS) for q in range(NQ)]
    # layer-2 pair slices: pair row = pgid >> 1, slice h covers q in (2h, 2h+1)
    q2rows = [
        (qrows[2 * h] + qrows[2 * h + 1] + 1) // 2 for h in range(NQ // 2)
    ]

    indeg = np.bincount(dst, minlength=N).astype(np.int64)
    dis = (1.0 / np.sqrt(indeg.astype(np.float64) + 1.0)).astype(np.float32)

    # snake assignment of degree-sorted nodes over TB blocks
    order = np.argsort(-indeg, kind="stable")
    i = np.arange(N)
    rnd = i // TB
    pos = i % TB
    blk_i = np.where(rnd % 2 == 0, pos, TB - 1 - pos)
    assert rnd.max() < P
    gid_of = np.empty(N, np.int64)
    gid_of[order] = blk_i * P + rnd

    # balance blocks over cores by slot count, then renumber into PHYSICAL
    # gid space: pgid = (core*NB + position)*128 + lane.  The AllGathered h2f
    # table is laid out in physical order, so gather indices, slice
    # bucketing, the xdis table, and outputs all use pgid.
    gdst0 = gid_of[dst] >> 7
    btot = np.bincount(gdst0, minlength=TB) + np.bincount(
        gid_of >> 7, minlength=TB
    )
    bord = np.argsort(-btot, kind="stable")
    block_of0 = np.empty((NCORES, NB), np.int64)
    for p in range(NB):
        row = bord[p * NCORES : (p + 1) * NCORES]
        if p % 2:
            row = row[::-1]
        block_of0[:, p] = row
    phys_pos = np.empty(TB, np.int64)
    for c in range(NCORES):
        for p in range(NB):
            phys_pos[block_of0[c, p]] = c * NB + p
    gid_of = phys_pos[gid_of >> 7] * P + (gid_of & 127)
    node_of_gid = np.full(NTOT, -1, np.int64)
    node_of_gid[gid_of] = np.arange(N)

    # slots = edges + self-loops, bucketed by (dst block, src slice, parity)
    all_src = np.concatenate([gid_of[src], gid_of])
    all_dst = np.concatenate([gid_of[dst], gid_of])
    sblk = all_dst >> 7
    slane = (all_dst & 127).astype(np.float32)
    sq = all_src // QS
    key2 = (sblk * NQ + sq) * 2 + (all_src & 1)
    ord2 = np.argsort(key2, kind="stable")
    k_src = all_src[ord2]
    k_lane = slane[ord2]
    cnt2 = np.bincount(key2, minlength=TB * NQ * 2).reshape(TB, NQ, 2)
    offs2 = np.zeros(TB * NQ * 2 + 1, np.int64)
    np.cumsum(cnt2.reshape(-1), out=offs2[1:])
    block_of = (np.arange(NCORES)[:, None] * NB) + np.arange(NB)[None, :]

    # uniform per-(position, slice, parity) sub-segment lengths
    seg_ev = cnt2[block_of, :, 0].max(axis=0)  # [NB, NQ]
    seg_od = cnt2[block_of, :, 1].max(axis=0)
    seg_len = seg_ev + seg_od

    # per-group call sizes + column schedules
    nch = np.zeros((NGRP, NQ), np.int64)
    oq = np.zeros((NGRP, NQ), np.int64)  # chunk offset of call q in group tile
    groups = []  # L1: per group, list of (p, [(q, ci, lo, hi)])
    groups2 = []  # L2: per group, list of (p, [(q, ci, par, lo, hi)])
    for g in range(NGRP):
        ps = list(range(g * GRP, (g + 1) * GRP))
        for q in range(NQ):
            L = int(seg_len[ps, q].sum())
            nch[g, q] = (L + P - 1) // P
        oq[g] = np.concatenate([[0], np.cumsum(nch[g])[:-1]])
        blocks = []
        blocks2 = []
        for bi, p in enumerate(ps):
            cols = []
            cols2 = []
            for q in range(NQ):
                t0 = int(seg_len[ps[:bi], q].sum())
                t1 = t0 + int(seg_len[p, q])
                if t1 == t0:
                    continue
                for ci in range(t0 // P, (t1 - 1) // P + 1):
                    cols.append((q, ci, t0, t1))
                for par, (lo, hi) in enumerate(
                    [
                        (t0, t0 + int(seg_ev[p, q])),
                        (t0 + int(seg_ev[p, q]), t1),
                    ]
                ):
                    if hi == lo:
                        continue
                    for ci in range(lo // P, (hi - 1) // P + 1):
                        cols2.append((q, ci, par, lo, hi))
            assert cols and cols2
            blocks.append((p, cols))
            blocks2.append((p, cols2))
        groups.append(blocks)
        groups2.append(blocks2)
    NCHG = int(nch.sum(1).max())
    TOTCOL = sum(len(c) for blks in groups for _, c in blks)
    TOTCOL2 = sum(len(c) for blks in groups2 for _, c in blks)

    # int16 idx column offsets: layer 1 per (g, q); layer 2 per (g, half)
    icol = np.zeros((NGRP, NQ), np.int64)
    run = 0
    for g in range(NGRP):
        for q in range(NQ):
            icol[g, q] = run
            run += int(nch[g, q]) * (P // 16)
    TOTICOL = int(run)
    icol2 = np.zeros((NGRP, NQ // 2), np.int64)
    run = 0
    for g in range(NGRP):
        for h in range(NQ // 2):
            icol2[g, h] = run
            run += int(nch[g, 2 * h] + nch[g, 2 * h + 1]) * (P // 16)
    assert int(run) == TOTICOL

    s.NB, s.TB, s.NPC, s.NTOT, s.NGRP, s.NQ = NB, TB, NPC, NTOT, NGRP, NQ
    s.qbase, s.qrows, s.q2rows = qbase, qrows, q2rows
    s.dis, s.gid_of, s.node_of_gid = dis, gid_of, node_of_gid
    s.k_src, s.k_lane, s.cnt2, s.offs2 = k_src, k_lane, cnt2, offs2
    s.block_of, s.seg_len, s.seg_ev = block_of, seg_len, seg_ev
    s.nch, s.oq, s.NCHG = nch, oq, NCHG
    s.groups, s.TOTCOL = groups, TOTCOL
    s.groups2, s.TOTCOL2 = groups2, TOTCOL2
    s.icol, s.icol2, s.TOTICOL = icol, icol2, TOTICOL
    return s


def _core_tensors(s, c):
    """Per-core dstl / dstl2 / esrc / esrc2 / disT arrays."""
    NB, NQ = s.NB, s.NQ

    esrc16 = np.zeros((P, s.TOTICOL), np.int16)
    esrc2 = np.zeros((P, s.TOTICOL), np.int16)
    dstl = np.full((P, s.TOTCOL + KB), PAD_LANE, np.float32)
    dstl2 = np.full((P, s.TOTCOL2 + KB), PAD_LANE, np.float32)

    # per (position, q): padded slot arrays (absolute src pgid + lanes)
    seg_src = {}
    seg_lane = {}
    for p in range(NB):
        tb = s.block_of[c, p]
        for q in range(NQ):
            L = int(s.seg_len[p, q])
            sv = np.full(L, s.qbase[q], np.int64)
            lv = np.full(L, PAD_LANE, np.float32)
            o0 = s.offs2[(tb * NQ + q) * 2]
            ne = int(s.cnt2[tb, q, 0])
            sv[:ne] = s.k_src[o0 : o0 + ne]
            lv[:ne] = s.k_lane[o0 : o0 + ne]
            o1 = s.offs2[(tb * NQ + q) * 2 + 1]
            no = int(s.cnt2[tb, q, 1])
            se = int(s.seg_ev[p, q])
            sv[se : se + no] = s.k_src[o1 : o1 + no]
            lv[se : se + no] = s.k_lane[o1 : o1 + no]
            seg_src[p, q] = sv
            seg_lane[p, q] = lv

    def wrap(seq):
        return np.tile(seq.reshape(-1, 16).T, (8, 1))

    j = 0
    j2 = 0
    for g in range(s.NGRP):
        ps = list(range(g * GRP, (g + 1) * GRP))
        # absolute src values per layer-1 call (incl. all pads)
        absq = {}
        for q in range(NQ):
            L = int(s.nch[g, q]) * P
            seq = np.full(L, s.qbase[q], np.int64)
            t = 0
            for p in ps:
                n = int(s.seg_len[p, q])
                seq[t : t + n] = seg_src[p, q]
                t += n
            absq[q] = seq
            io = s.icol[g, q]
            esrc16[:, io : io + L // 16] = wrap(
                (seq - s.qbase[q]).astype(np.int16)
            )
        for h in range(NQ // 2):
            seq = np.concatenate([absq[2 * h], absq[2 * h + 1]])
            io = s.icol2[g, h]
            esrc2[:, io : io + len(seq) // 16] = wrap(
                ((seq >> 1) - h * QS).astype(np.int16)
            )
        # mask columns (emission order: block-major within group)
        for p, cols in s.groups[g]:
            for q, ci, t0, _t1 in cols:
                lv = seg_lane[p, q]
                lo = max(ci * P, t0)
                hi = min(ci * P + P, t0 + len(lv))
                if hi > lo:
                    dstl[lo - ci * P : hi - ci * P, j] = lv[lo - t0 : hi - t0]
                j += 1
        for p, cols in s.groups2[g]:
            for q, ci, par, lo_s, hi_s in cols:
                lv = seg_lane[p, q]
                t0 = lo_s if par == 0 else lo_s - int(s.seg_ev[p, q])
                lo = max(ci * P, lo_s)
                hi = min(ci * P + P, hi_s)
                if hi > lo:
                    dstl2[lo - ci * P : hi - ci * P, j2] = lv[
                        lo - t0 : hi - t0
                    ]
                j2 += 1
    assert j == s.TOTCOL and j2 == s.TOTCOL2

    gids = s.block_of[c][:, None] * P + np.arange(P)[None, :]  # [NB, P]
    nodes = s.node_of_gid[gids]
    disT = np.ones((P, NB), np.float32)
    valid = nodes >= 0
    disT.T[valid] = s.dis[nodes[valid]]

    return (
        np.ascontiguousarray(esrc16),
        np.ascontiguousarray(esrc2),
        np.ascontiguousarray(dstl.astype(ml_dtypes.bfloat16)),
        np.ascontiguousarray(dstl2.astype(ml_dtypes.bfloat16)),
        np.ascontiguousarray(disT),
        nodes,
    )


# --------------------------------------------------------------------------
# Device program
# --------------------------------------------------------------------------


def _build_program(s, IN_CH, HID, OUT_CH, has_b1, has_b2):
    NB, NQ, NCHG, NGRP = s.NB, s.NQ, s.NCHG, s.NGRP
    KT = IN_CH // P
    assert IN_CH % P == 0 and HID == P and OUT_CH * 2 == P

    nc = bass.Bass()
    xdis = nc.dram_tensor("xdis", [s.NTOT, IN_CH], BF16, kind="ExternalInput")
    W1 = nc.dram_tensor("W1", [IN_CH, HID], BF16, kind="ExternalInput")
    W2c = nc.dram_tensor("W2c", [HID, OUT_CH], BF16, kind="ExternalInput")
    b1bc = nc.dram_tensor("b1bc", [P, HID], F32, kind="ExternalInput")
    b2bc = nc.dram_tensor("b2bc", [P, OUT_CH], F32, kind="ExternalInput")
    disT = nc.dram_tensor("disT", [P, NB], F32, kind="ExternalInput")
    iotaK = nc.dram_tensor("iotaK", [P, P * KB], BF16, kind="ExternalInput")
    esrc = nc.dram_tensor("esrc", [P, s.TOTICOL], I16, kind="ExternalInput")
    esrc2t = nc.dram_tensor("esrc2", [P, s.TOTICOL], I16, kind="ExternalInput")
    dstl = nc.dram_tensor("dstl", [P, s.TOTCOL + KB], BF16, kind="ExternalInput")
    dstl2t = nc.dram_tensor(
        "dstl2", [P, s.TOTCOL2 + KB], BF16, kind="ExternalInput"
    )
    outY = nc.dram_tensor("outY", [s.NPC, OUT_CH], F32, kind="ExternalOutput")

    h2s = nc.dram_tensor("h2s", [s.NPC, OUT_CH], BF16)
    h2f = nc.dram_tensor("h2f", [s.NTOT, OUT_CH], BF16)

    rg = [list(range(NCORES))]
    RELU = mybir.ActivationFunctionType.Relu
    ADD = mybir.AluOpType.add
    ISEQ = mybir.AluOpType.is_equal

    with tile.TileContext(nc) as tc:
        with tc.tile_pool(name="const", bufs=1) as cst:
            iotaKsb = cst.tile([P, P * KB], BF16)
            nc.sync.dma_start(out=iotaKsb[:], in_=iotaK[:, :])
            idsb = cst.tile([P, P], BF16)
            make_identity(nc, idsb[:])
            # custom-op ucode (dma_gather) — after the stock gpsimd ops above
            nc.gpsimd.load_library(library_config.mlp)
            nvals = {int(v) * P for v in np.unique(s.nch) if v}
            nvals |= {
                int(s.nch[g, 2 * h] + s.nch[g, 2 * h + 1]) * P
                for g in range(NGRP)
                for h in range(NQ // 2)
            }
            nidx_regs = {}
            for v in sorted(nvals):
                nidx_regs[v] = nc.gpsimd.to_reg(v)

            w1sb = cst.tile([P, KT * HID], BF16)
            for k in range(KT):
                nc.sync.dma_start(
                    out=w1sb[:, k * HID : (k + 1) * HID],
                    in_=W1[k * P : (k + 1) * P, :],
                )
            w2sb = cst.tile([P, OUT_CH], BF16)
            nc.sync.dma_start(out=w2sb[:], in_=W2c[:, :])
            dissb = cst.tile([P, NB], F32)
            nc.sync.dma_start(out=dissb[:], in_=disT[:, :])
            esrcsb = cst.tile([P, s.TOTICOL], I16)
            nc.sync.dma_start(out=esrcsb[:], in_=esrc[:, :])
            dstlsb = cst.tile([P, s.TOTCOL + KB], BF16)
            nc.sync.dma_start(out=dstlsb[:], in_=dstl[:, :])
            dstl2sb = cst.tile([P, s.TOTCOL2 + KB], BF16)
            nc.sync.dma_start(out=dstl2sb[:], in_=dstl2t[:, :])
            if has_b1:
                b1sb = cst.tile([P, HID], F32)
                nc.sync.dma_start(out=b1sb[:], in_=b1bc[:, :])
            if has_b2:
                b2sb = cst.tile([P, OUT_CH], F32)
                nc.sync.dma_start(out=b2sb[:], in_=b2bc[:, :])

            # ---------------- layer 1: aggregate xdis, emit h2 table -------
            with (
                tc.tile_pool(name="g1", bufs=2) as pg,
                tc.tile_pool(name="m1", bufs=6) as pm,
                tc.tile_pool(name="s1", bufs=3) as psb,
                tc.tile_pool(name="z1", bufs=3, space="PSUM") as pz,
                tc.tile_pool(name="t1", bufs=2, space="PSUM") as pt,
            ):

                def _l1_epilogue(p, zt):
                    zsb = psb.tile([P, IN_CH], BF16, tag="zsb")
                    nc.scalar.copy(out=zsb[:], in_=zt[:])
                    zt_ts = psb.tile([P, KT * P], BF16, tag="zts")
                    for k in range(KT):
                        tr = pt.tile([P, P], BF16, tag="tr")
                        nc.tensor.transpose(
                            out=tr[:],
                            in_=zsb[:, k * P : (k + 1) * P],
                            identity=idsb[:],
                        )
                        nc.scalar.copy(
                            out=zt_ts[:, k * P : (k + 1) * P], in_=tr[:]
                        )
                    o1ps = pt.tile([P, HID], F32, tag="tp")
                    for k in range(KT):
                        nc.tensor.matmul(
                            o1ps[:],
                            lhsT=zt_ts[:, k * P : (k + 1) * P],
                            rhs=w1sb[:, k * HID : (k + 1) * HID],
                            start=(k == 0),
                            stop=(k == KT - 1),
                        )
                    o1sb = psb.tile([P, HID], BF16, tag="o1")
                    if has_b1:
                        t0f = psb.tile([P, HID], F32, tag="t0f")
                        nc.vector.tensor_scalar_mul(
                            t0f[:], o1ps[:], dissb[:, p : p + 1]
                        )
                        nc.vector.tensor_tensor(t0f[:], t0f[:], b1sb[:], op=ADD)
                        nc.scalar.activation(o1sb[:], t0f[:], RELU)
                    else:
                        nc.scalar.activation(
                            o1sb[:], o1ps[:], RELU, scale=dissb[:, p : p + 1]
                        )
                    trp = pt.tile([P, HID], BF16, tag="tr")
                    nc.tensor.transpose(out=trp[:], in_=o1sb[:], identity=idsb[:])
                    o1t = psb.tile([P, HID], BF16, tag="o1t")
                    nc.scalar.copy(out=o1t[:], in_=trp[:])
                    h2ps = pt.tile([P, P], F32, tag="tp")
                    nc.tensor.matmul(
                        h2ps[:, :OUT_CH],
                        lhsT=o1t[:],
                        rhs=w2sb[:],
                        start=True,
                        stop=True,
                    )
                    h2sb = psb.tile([P, OUT_CH], BF16, tag="h2")
                    nc.vector.tensor_scalar_mul(
                        h2sb[:], h2ps[:, :OUT_CH], dissb[:, p : p + 1]
                    )
                    nc.sync.dma_start(
                        out=h2s[p * P : (p + 1) * P, :], in_=h2sb[:]
                    )

                pending = None
                j = 0
                for g in range(NGRP):
                    gt = pg.tile([P, NCHG, IN_CH], BF16, tag="gt")
                    for q in range(NQ):
                        nq = int(s.nch[g, q])
                        if nq == 0:
                            continue
                        o = int(s.oq[g, q])
                        io = int(s.icol[g, q])
                        nidx = nq * P
                        nc.gpsimd.dma_gather(
                            gt[:, o : o + nq, :],
                            xdis[s.qbase[q] : s.qbase[q] + s.qrows[q], :],
                            esrcsb[:, io : io + nq * (P // 16)],
                            nidx,
                            nidx_regs[nidx],
                            IN_CH,
                            single_packet=False,
                        )
                    j0c = j
                    cig = 0
                    cur = None
                    for p, cols in s.groups[g]:
                        zt = pz.tile([P, IN_CH], F32, tag="z")
                        ncol = len(cols)
                        for ki, (q, ci, _t0, _t1) in enumerate(cols):
                            bb, r = divmod(cig, KB)
                            if r == 0:
                                cur = pm.tile([P, P * KB], BF16, tag="m")
                                nc.vector.tensor_tensor(
                                    cur[:].rearrange("p (l k) -> p l k", k=KB),
                                    iotaKsb[:].rearrange(
                                        "p (l k) -> p l k", k=KB
                                    ),
                                    dstlsb[:, j0c + bb * KB : j0c + (bb + 1) * KB]
                                    .unsqueeze(1)
                                    .broadcast_to([P, P, KB]),
                                    op=ISEQ,
                                )
                            cig += 1
                            j += 1
                            mv = cur[:].rearrange("p (l k) -> p k l", k=KB)
                            nc.tensor.matmul(
                                zt[:],
                                lhsT=mv[:, r, :],
                                rhs=gt[:, int(s.oq[g, q]) + ci, :],
                                start=(ki == 0),
                                stop=(ki == ncol - 1),
                            )
                        # software pipeline: emit the PREVIOUS block's dense
                        # epilogue now, so its PE ops never stall on ACT
                        # round-trips (next block's agg matmuls fill the gap).
                        if pending is not None:
                            pending()
                        pending = (lambda p=p, zt=zt: _l1_epilogue(p, zt))
                        continue

                if pending is not None:
                    pending()
                    pending = None
                assert j == s.TOTCOL

            # ---------------- AllGather of the compact h2 message table ----
            nc.gpsimd.collective_compute(
                "AllGather",
                mybir.AluOpType.bypass,
                replica_groups=rg,
                ins=[h2s[:, :]],
                outs=[h2f[:, :]],
            )

            # ---------------- layer 2: pair-gather h2, write output --------
            h2fp = h2f[:, :].rearrange("(r two) f -> r (two f)", two=2)
            with (
                tc.tile_pool(name="g2", bufs=2) as pg2,
                tc.tile_pool(name="m2", bufs=6) as pm2,
                tc.tile_pool(name="s2", bufs=3) as psb2,
                tc.tile_pool(name="z2", bufs=2, space="PSUM") as pz2,
                tc.tile_pool(name="e2p", bufs=1) as pe2,
            ):
                esrc2sb = pe2.tile([P, s.TOTICOL], I16)
                nc.sync.dma_start(out=esrc2sb[:], in_=esrc2t[:, :])
                j = 0
                for g in range(NGRP):
                    gt = pg2.tile([P, NCHG, P], BF16, tag="gt")
                    for h in range(NQ // 2):
                        nq = int(s.nch[g, 2 * h] + s.nch[g, 2 * h + 1])
                        if nq == 0:
                            continue
                        o = int(s.oq[g, 2 * h])
                        io = int(s.icol2[g, h])
                        nidx = nq * P
                        nc.gpsimd.dma_gather(
                            gt[:, o : o + nq, :],
                            h2fp[h * QS : h * QS + s.q2rows[h], :],
                            esrc2sb[:, io : io + nq * (P // 16)],
                            nidx,
                            nidx_regs[nidx],
                            P,
                            single_packet=False,
                        )
                    j0c = j
                    cig = 0
                    cur = None
                    for p, cols in s.groups2[g]:
                        zt = pz2.tile([P, OUT_CH], F32, tag="z")
                        ncol = len(cols)
                        for ki, (q, ci, par, _lo, _hi) in enumerate(cols):
                            bb, r = divmod(cig, KB)
                            if r == 0:
                                cur = pm2.tile([P, P * KB], BF16, tag="m")
                                nc.vector.tensor_tensor(
                                    cur[:].rearrange("p (l k) -> p l k", k=KB),
                                    iotaKsb[:].rearrange(
                                        "p (l k) -> p l k", k=KB
                                    ),
                                    dstl2sb[
                                        :, j0c + bb * KB : j0c + (bb + 1) * KB
                                    ]
                                    .unsqueeze(1)
                                    .broadcast_to([P, P, KB]),
                                    op=ISEQ,
                                )
                            cig += 1
                            j += 1
                            mv = cur[:].rearrange("p (l k) -> p k l", k=KB)
                            nc.tensor.matmul(
                                zt[:],
                                lhsT=mv[:, r, :],
                                rhs=gt[
                                    :,
                                    int(s.oq[g, q]) + ci,
                                    par * OUT_CH : (par + 1) * OUT_CH,
                                ],
                                start=(ki == 0),
                                stop=(ki == ncol - 1),
                            )
                        osb = psb2.tile([P, OUT_CH], F32, tag="o")
                        if has_b2:
                            t2f = psb2.tile([P, OUT_CH], F32, tag="t2f")
                            nc.vector.tensor_scalar_mul(
                                t2f[:], zt[:], dissb[:, p : p + 1]
                            )
                            nc.vector.tensor_tensor(t2f[:], t2f[:], b2sb[:], op=ADD)
                            nc.scalar.activation(osb[:], t2f[:], RELU)
                        else:
                            nc.scalar.activation(
                                osb[:], zt[:], RELU, scale=dissb[:, p : p + 1]
                            )
                        nc.sync.dma_start(
                            out=outY[p * P : (p + 1) * P, :], in_=osb[:]
                        )
                assert j == s.TOTCOL2

    _spill_waits(nc)
    mybir.codegen_inst_isa_subclasses(nc)
    return nc


# --------------------------------------------------------------------------
# Entry point
# --------------------------------------------------------------------------


def kernel(x, edge_index, W1, b1, W2, b2):
    x = np.asarray(x, dtype=np.float32)
    W1 = np.asarray(W1, dtype=np.float32)
    W2 = np.asarray(W2, dtype=np.float32)
    b1 = np.asarray(b1, dtype=np.float32)
    b2 = np.asarray(b2, dtype=np.float32)
    src = np.asarray(edge_index[0]).astype(np.int64)
    dst = np.asarray(edge_index[1]).astype(np.int64)

    N, IN_CH = x.shape
    HID = W1.shape[1]
    OUT_CH = W2.shape[1]

    s = _prepare(N, src, dst)

    # full xdis replica, pgid-indexed (empty gids = 0)
    xdis = np.zeros((s.NTOT, IN_CH), ml_dtypes.bfloat16)
    xdis[s.gid_of] = (x * s.dis[:, None]).astype(ml_dtypes.bfloat16)

    W1b = np.ascontiguousarray(W1.astype(ml_dtypes.bfloat16))
    W2b = np.ascontiguousarray(W2.astype(ml_dtypes.bfloat16))
    has_b1 = bool(np.any(b1))
    has_b2 = bool(np.any(b2))
    b1bc = np.ascontiguousarray(np.broadcast_to(b1, (P, HID)).astype(np.float32))
    b2bc = np.ascontiguousarray(np.broadcast_to(b2, (P, OUT_CH)).astype(np.float32))

    nc = _build_program(s, IN_CH, HID, OUT_CH, has_b1, has_b2)

    iotaK = np.zeros((P, P * KB), ml_dtypes.bfloat16)
    for l in range(P):
        iotaK[:, l * KB : (l + 1) * KB] = l

    in_maps = []
    node_maps = []
    for c in range(NCORES):
        esrc16, esrc2, dstlc, dstl2c, disTc, nodes = _core_tensors(s, c)
        node_maps.append(nodes)
        in_maps.append(
            {
                "xdis": xdis,
                "W1": W1b,
                "W2c": W2b,
                "b1bc": b1bc,
                "b2bc": b2bc,
                "disT": disTc,
                "iotaK": iotaK,
                "esrc": esrc16,
                "esrc2": esrc2,
                "dstl": dstlc,
                "dstl2": dstl2c,
            }
        )

    res = run_bass_kernel_spmd(nc, in_maps, core_ids=list(range(NCORES)))
    global _last_results, _last_nc
    _last_results = res
    _last_nc = nc

    out = np.empty((N, OUT_CH), np.float32)
    for c in range(NCORES):
        oc = res.results[c]["outY"]  # [NPC, OUT_CH], row = p*128+lane
        nodes = node_maps[c].reshape(-1)  # [NB*P] original node or -1
        valid = nodes >= 0
        out[nodes[valid]] = oc[valid]
    return out


# revision 14
# speedup vs baseline: 3.2278x; 1.0152x over previous
"""2-layer GCN (PyG GCNConv x2 + ReLU) on 8 Trainium2 NeuronCores.

Math per layer (A from edge_index, deg = indeg(dst)+1, dis = deg^-1/2):
    out[d] = relu( dis_d * ( sum_{e: s->d} dis_s*h[s] + dis_d*h[d] ) @ W + b )

Strategy:
  * Layer 1 is aggregate-then-transform: every core holds a full replica of
    xdis = x*dis (bf16) in its HBM (NO layer-1 collective).  Each core
    aggregates Z = sum xdis[src] for its own destination blocks with large
    dma_gather calls + one-hot-mask matmuls, then applies W1 and W2 densely
    per 128-node block, producing the layer-2 message table h2 = dis*(relu@W2).
  * One AllGather shares the compact [NTOT, 64] bf16 h2 table.
  * Layer 2 gathers PAIRS of h2 rows (256B descriptors cover two nodes ->
    half the per-edge DMA cost); slots are parity-sorted inside each
    (block, slice) segment so each mask column selects one 64-col half.
  * Self-loops are folded in as ordinary edges (src == dst).
  * dma_gather uses int16 indices: the node table is split in 4 row slices
    of <=32768 (layer 1) / 2 pair-slices (layer 2).  The slot schedule is
    made identical across cores by padding each (block, slice, parity)
    sub-segment to the max over cores; only tensor data differs per core.
  * One-hot masks are built 8 columns per DVE instruction (interleaved iota
    constant + broadcast-AP tensor_tensor is_equal); the PE reads the mask
    slices with a strided lhsT access pattern.
"""

import math

import ml_dtypes
import numpy as np

import concourse.bass as bass
import concourse.mybir as mybir
import concourse.tile as tile
from concourse import library_config
from concourse.bass_utils import run_bass_kernel_spmd
from concourse.masks import make_identity
from concourse.vector_clock import ScopedClock

P = 128
NCORES = 8
PAD_LANE = 1000.0  # dst-lane sentinel for padding edge slots (one-hot = 0)
QS = 32768  # dma_gather int16 index range -> table row-slice size
GRP = 7  # dst blocks per gather group
KB = 8  # one-hot mask columns built per DVE instruction
BUFS_G1, BUFS_M1, BUFS_Z1 = 2, 6, 3
BUFS_G2, BUFS_M2, BUFS_Z2 = 2, 6, 6

F32 = mybir.dt.float32
BF16 = mybir.dt.bfloat16
I16 = mybir.dt.int16


def _patched_drain_and_barrier(self, tick_clock, wait_clock):
    # This walrus build rejects >1 sem wait on TPB_CTRL (Drain) instructions.
    # Spill the tile-epilogue drain waits onto extra single-wait drains.
    drain_inst = self.nc.sync.drain()
    wait_clock.add_sem_waits(
        drain_inst.ins, ScopedClock({None: tick_clock.global_clock})
    )
    si = drain_inst.ins.sync_info
    waits = list(si.on_wait)
    if len(waits) > 1:
        while len(si.on_wait):
            si.on_wait.pop()
        si.on_wait.append(waits[0])
        for w in waits[1:]:
            d2 = self.nc.sync.drain(fusable=False)
            si2 = d2.ins.sync_info
            if si2 is None:
                d2.ins.sync_info = mybir.SyncInfo(on_wait=[w], on_update=[])
            else:
                si2.on_wait.append(w)
    self.nc.all_engine_barrier()
    popped = self.nc._tile_sem_poison_stack.pop()
    assert popped is self._sem_poison
    self.nc.clear_and_free_semaphores(list(self.sems.allocated().values()))
    self.nc.all_engine_barrier()


tile.TileContext._drain_and_barrier = _patched_drain_and_barrier


def _spill_waits(nc, max_waits=1):
    """This walrus build accepts at most one sync wait per instruction.
    Move extra waits onto dedicated single-wait NoOps ahead of the
    instruction on the same engine (engines execute in program order)."""
    n = 0
    for f in nc.m.functions:
        for blk in f.blocks:
            il = blk.instructions
            out = []
            for inst in il:
                si = inst.sync_info
                if si is not None and len(si.on_wait) > max_waits:
                    waits = list(si.on_wait)
                    while len(si.on_wait):
                        si.on_wait.pop()
                    for w in waits[:max_waits]:
                        si.on_wait.append(w)
                    for w in waits[max_waits:]:
                        nop = mybir.InstNoOp(
                            name=f"waitspill-{n}",
                            sync_info=mybir.SyncInfo(on_wait=[w], on_update=[]),
                            bass_nofuse=True,
                            engine=inst.engine,
                        )
                        n += 1
                        out.append(nop)
                out.append(inst)
            blk.instructions = out
    return n


# --------------------------------------------------------------------------
# Host-side schedule construction
# --------------------------------------------------------------------------


class _Sched:
    pass


def _prepare(N, src, dst):
    """Node->gid assignment, slot buckets, and the (core-uniform) gather /
    column schedule for both layers."""
    s = _Sched()
    NB = int(math.ceil(N / (NCORES * P)))
    TB = NB * NCORES
    NPC = NB * P
    NTOT = TB * P
    NGRP = int(math.ceil(NB / GRP))
    assert NGRP * GRP == NB, (NB, NGRP)
    NQ = int(math.ceil(NTOT / QS))
    assert NQ % 2 == 0
    qbase = [q * QS for q in range(NQ)]
    qrows = [min(QS, NTOT - q * QS) for q in range(NQ)]
    # layer-2 pair slices: pair row = pgid >> 1, slice h covers q in (2h, 2h+1)
    q2rows = [
        (qrows[2 * h] + qrows[2 * h + 1] + 1) // 2 for h in range(NQ // 2)
    ]

    indeg = np.bincount(dst, minlength=N).astype(np.int64)
    dis = (1.0 / np.sqrt(indeg.astype(np.float64) + 1.0)).astype(np.float32)

    # snake assignment of degree-sorted nodes over TB blocks
    order = np.argsort(-indeg, kind="stable")
    i = np.arange(N)
    rnd = i // TB
    pos = i % TB
    blk_i = np.where(rnd % 2 == 0, pos, TB - 1 - pos)
    assert rnd.max() < P
    gid_of = np.empty(N, np.int64)
    gid_of[order] = blk_i * P + rnd

    # balance blocks over cores by slot count, then renumber into PHYSICAL
    # gid space: pgid = (core*NB + position)*128 + lane.  The AllGathered h2f
    # table is laid out in physical order, so gather indices, slice
    # bucketing, the xdis table, and outputs all use pgid.
    gdst0 = gid_of[dst] >> 7
    btot = np.bincount(gdst0, minlength=TB) + np.bincount(
        gid_of >> 7, minlength=TB
    )
    bord = np.argsort(-btot, kind="stable")
    block_of0 = np.empty((NCORES, NB), np.int64)
    for p in range(NB):
        row = bord[p * NCORES : (p + 1) * NCORES]
        if p % 2:
            row = row[::-1]
        block_of0[:, p] = row
    phys_pos = np.empty(TB, np.int64)
    for c in range(NCORES):
        for p in range(NB):
            phys_pos[block_of0[c, p]] = c * NB + p
    gid_of = phys_pos[gid_of >> 7] * P + (gid_of & 127)
    node_of_gid = np.full(NTOT, -1, np.int64)
    node_of_gid[gid_of] = np.arange(N)

    # slots = edges + self-loops, bucketed by (dst block, src slice, parity)
    all_src = np.concatenate([gid_of[src], gid_of])
    all_dst = np.concatenate([gid_of[dst], gid_of])
    sblk = all_dst >> 7
    slane = (all_dst & 127).astype(np.float32)
    sq = all_src // QS
    key2 = (sblk * NQ + sq) * 2 + (all_src & 1)
    ord2 = np.argsort(key2, kind="stable")
    k_src = all_src[ord2]
    k_lane = slane[ord2]
    cnt2 = np.bincount(key2, minlength=TB * NQ * 2).reshape(TB, NQ, 2)
    offs2 = np.zeros(TB * NQ * 2 + 1, np.int64)
    np.cumsum(cnt2.reshape(-1), out=offs2[1:])
    block_of = (np.arange(NCORES)[:, None] * NB) + np.arange(NB)[None, :]

    # uniform per-(position, slice, parity) sub-segment lengths
    seg_ev = cnt2[block_of, :, 0].max(axis=0)  # [NB, NQ]
    seg_od = cnt2[block_of, :, 1].max(axis=0)
    seg_len = seg_ev + seg_od

    # per-group call sizes + column schedules
    nch = np.zeros((NGRP, NQ), np.int64)
    oq = np.zeros((NGRP, NQ), np.int64)  # chunk offset of call q in group tile
    groups = []  # L1: per group, list of (p, [(q, ci, lo, hi)])
    groups2 = []  # L2: per group, list of (p, [(q, ci, par, lo, hi)])
    for g in range(NGRP):
        ps = list(range(g * GRP, (g + 1) * GRP))
        for q in range(NQ):
            L = int(seg_len[ps, q].sum())
            nch[g, q] = (L + P - 1) // P
        oq[g] = np.concatenate([[0], np.cumsum(nch[g])[:-1]])
        blocks = []
        blocks2 = []
        for bi, p in enumerate(ps):
            cols = []
            cols2 = []
            for q in range(NQ):
                t0 = int(seg_len[ps[:bi], q].sum())
                t1 = t0 + int(seg_len[p, q])
                if t1 == t0:
                    continue
                for ci in range(t0 // P, (t1 - 1) // P + 1):
                    cols.append((q, ci, t0, t1))
                for par, (lo, hi) in enumerate(
                    [
                        (t0, t0 + int(seg_ev[p, q])),
                        (t0 + int(seg_ev[p, q]), t1),
                    ]
                ):
                    if hi == lo:
                        continue
                    for ci in range(lo // P, (hi - 1) // P + 1):
                        cols2.append((q, ci, par, lo, hi))
            assert cols and cols2
            blocks.append((p, cols))
            blocks2.append((p, cols2))
        groups.append(blocks)
        groups2.append(blocks2)
    NCHG = int(nch.sum(1).max())
    TOTCOL = sum(len(c) for blks in groups for _, c in blks)
    TOTCOL2 = sum(len(c) for blks in groups2 for _, c in blks)

    # int16 idx column offsets: layer 1 per (g, q); layer 2 per (g, half)
    icol = np.zeros((NGRP, NQ), np.int64)
    run = 0
    for g in range(NGRP):
        for q in range(NQ):
            icol[g, q] = run
            run += int(nch[g, q]) * (P // 16)
    TOTICOL = int(run)
    icol2 = np.zeros((NGRP, NQ // 2), np.int64)
    run = 0
    for g in range(NGRP):
        for h in range(NQ // 2):
            icol2[g, h] = run
            run += int(nch[g, 2 * h] + nch[g, 2 * h + 1]) * (P // 16)
    assert int(run) == TOTICOL

    s.NB, s.TB, s.NPC, s.NTOT, s.NGRP, s.NQ = NB, TB, NPC, NTOT, NGRP, NQ
    s.qbase, s.qrows, s.q2rows = qbase, qrows, q2rows
    s.dis, s.gid_of, s.node_of_gid = dis, gid_of, node_of_gid
    s.k_src, s.k_lane, s.cnt2, s.offs2 = k_src, k_lane, cnt2, offs2
    s.block_of, s.seg_len, s.seg_ev = block_of, seg_len, seg_ev
    s.nch, s.oq, s.NCHG = nch, oq, NCHG
    s.groups, s.TOTCOL = groups, TOTCOL
    s.groups2, s.TOTCOL2 = groups2, TOTCOL2
    s.icol, s.icol2, s.TOTICOL = icol, icol2, TOTICOL
    return s


def _core_tensors(s, c):
    """Per-core dstl / dstl2 / esrc / esrc2 / disT arrays."""
    NB, NQ = s.NB, s.NQ

    esrc16 = np.zeros((P, s.TOTICOL), np.int16)
    esrc2 = np.zeros((P, s.TOTICOL), np.int16)
    dstl = np.full((P, s.TOTCOL + KB), PAD_LANE, np.float32)
    dstl2 = np.full((P, s.TOTCOL2 + KB), PAD_LANE, np.float32)

    # per (position, q): padded slot arrays (absolute src pgid + lanes)
    seg_src = {}
    seg_lane = {}
    for p in range(NB):
        tb = s.block_of[c, p]
        for q in range(NQ):
            L = int(s.seg_len[p, q])
            sv = np.full(L, s.qbase[q], np.int64)
            lv = np.full(L, PAD_LANE, np.float32)
            o0 = s.offs2[(tb * NQ + q) * 2]
            ne = int(s.cnt2[tb, q, 0])
            sv[:ne] = s.k_src[o0 : o0 + ne]
            lv[:ne] = s.k_lane[o0 : o0 + ne]
            o1 = s.offs2[(tb * NQ + q) * 2 + 1]
            no = int(s.cnt2[tb, q, 1])
            se = int(s.seg_ev[p, q])
            sv[se : se + no] = s.k_src[o1 : o1 + no]
            lv[se : se + no] = s.k_lane[o1 : o1 + no]
            seg_src[p, q] = sv
            seg_lane[p, q] = lv

    def wrap(seq):
        return np.tile(seq.reshape(-1, 16).T, (8, 1))

    j = 0
    j2 = 0
    for g in range(s.NGRP):
        ps = list(range(g * GRP, (g + 1) * GRP))
        # absolute src values per layer-1 call (incl. all pads)
        absq = {}
        for q in range(NQ):
            L = int(s.nch[g, q]) * P
            seq = np.full(L, s.qbase[q], np.int64)
            t = 0
            for p in ps:
                n = int(s.seg_len[p, q])
                seq[t : t + n] = seg_src[p, q]
                t += n
            absq[q] = seq
            io = s.icol[g, q]
            esrc16[:, io : io + L // 16] = wrap(
                (seq - s.qbase[q]).astype(np.int16)
            )
        for h in range(NQ // 2):
            seq = np.concatenate([absq[2 * h], absq[2 * h + 1]])
            io = s.icol2[g, h]
            esrc2[:, io : io + len(seq) // 16] = wrap(
                ((seq >> 1) - h * QS).astype(np.int16)
            )
        # mask columns (emission order: block-major within group)
        for p, cols in s.groups[g]:
            for q, ci, t0, _t1 in cols:
                lv = seg_lane[p, q]
                lo = max(ci * P, t0)
                hi = min(ci * P + P, t0 + len(lv))
                if hi > lo:
                    dstl[lo - ci * P : hi - ci * P, j] = lv[lo - t0 : hi - t0]
                j += 1
        for p, cols in s.groups2[g]:
            for q, ci, par, lo_s, hi_s in cols:
                lv = seg_lane[p, q]
                t0 = lo_s if par == 0 else lo_s - int(s.seg_ev[p, q])
                lo = max(ci * P, lo_s)
                hi = min(ci * P + P, hi_s)
                if hi > lo:
                    dstl2[lo - ci * P : hi - ci * P, j2] = lv[
                        lo - t0 : hi - t0
                    ]
                j2 += 1
    assert j == s.TOTCOL and j2 == s.TOTCOL2

    gids = s.block_of[c][:, None] * P + np.arange(P)[None, :]  # [NB, P]
    nodes = s.node_of_gid[gids]
    disT = np.ones((P, NB), np.float32)
    valid = nodes >= 0
    disT.T[valid] = s.dis[nodes[valid]]

    return (
        np.ascontiguousarray(esrc16),
        np.ascontiguousarray(esrc2),
        np.ascontiguousarray(dstl.astype(ml_dtypes.bfloat16)),
        np.ascontiguousarray(dstl2.astype(ml_dtypes.bfloat16)),
        np.ascontiguousarray(disT),
        nodes,
    )


# --------------------------------------------------------------------------
# Device program
# --------------------------------------------------------------------------


def _build_program(s, IN_CH, HID, OUT_CH, has_b1, has_b2):
    NB, NQ, NCHG, NGRP = s.NB, s.NQ, s.NCHG, s.NGRP
    KT = IN_CH // P
    assert IN_CH % P == 0 and HID == P and OUT_CH * 2 == P

    nc = bass.Bass()
    xdis = nc.dram_tensor("xdis", [s.NTOT, IN_CH], BF16, kind="ExternalInput")
    W1 = nc.dram_tensor("W1", [IN_CH, HID], BF16, kind="ExternalInput")
    W2c = nc.dram_tensor("W2c", [HID, OUT_CH], BF16, kind="ExternalInput")
    b1bc = nc.dram_tensor("b1bc", [P, HID], F32, kind="ExternalInput")
    b2bc = nc.dram_tensor("b2bc", [P, OUT_CH], F32, kind="ExternalInput")
    disT = nc.dram_tensor("disT", [P, NB], F32, kind="ExternalInput")
    iotaK = nc.dram_tensor("iotaK", [P, P * KB], BF16, kind="ExternalInput")
    esrc = nc.dram_tensor("esrc", [P, s.TOTICOL], I16, kind="ExternalInput")
    esrc2t = nc.dram_tensor("esrc2", [P, s.TOTICOL], I16, kind="ExternalInput")
    dstl = nc.dram_tensor("dstl", [P, s.TOTCOL + KB], BF16, kind="ExternalInput")
    dstl2t = nc.dram_tensor(
        "dstl2", [P, s.TOTCOL2 + KB], BF16, kind="ExternalInput"
    )
    outY = nc.dram_tensor("outY", [s.NPC, OUT_CH], F32, kind="ExternalOutput")

    h2s = nc.dram_tensor("h2s", [s.NPC, OUT_CH], BF16)
    h2f = nc.dram_tensor("h2f", [s.NTOT, OUT_CH], BF16)

    rg = [list(range(NCORES))]
    RELU = mybir.ActivationFunctionType.Relu
    ADD = mybir.AluOpType.add
    ISEQ = mybir.AluOpType.is_equal

    with tile.TileContext(nc) as tc:
        with tc.tile_pool(name="const", bufs=1) as cst:
            iotaKsb = cst.tile([P, P * KB], BF16)
            nc.sync.dma_start(out=iotaKsb[:], in_=iotaK[:, :])
            idsb = cst.tile([P, P], BF16)
            make_identity(nc, idsb[:])
            # custom-op ucode (dma_gather) — after the stock gpsimd ops above
            nc.gpsimd.load_library(library_config.mlp)
            nvals = {int(v) * P for v in np.unique(s.nch) if v}
            nvals |= {
                int(s.nch[g, 2 * h] + s.nch[g, 2 * h + 1]) * P
                for g in range(NGRP)
                for h in range(NQ // 2)
            }
            nidx_regs = {}
            for v in sorted(nvals):
                nidx_regs[v] = nc.gpsimd.to_reg(v)

            w1sb = cst.tile([P, KT * HID], BF16)
            for k in range(KT):
                nc.sync.dma_start(
                    out=w1sb[:, k * HID : (k + 1) * HID],
                    in_=W1[k * P : (k + 1) * P, :],
                )
            w2sb = cst.tile([P, OUT_CH], BF16)
            nc.sync.dma_start(out=w2sb[:], in_=W2c[:, :])
            dissb = cst.tile([P, NB], F32)
            nc.sync.dma_start(out=dissb[:], in_=disT[:, :])
            esrcsb = cst.tile([P, s.TOTICOL], I16)
            nc.sync.dma_start(out=esrcsb[:], in_=esrc[:, :])
            dstlsb = cst.tile([P, s.TOTCOL + KB], BF16)
            nc.sync.dma_start(out=dstlsb[:], in_=dstl[:, :])
            dstl2sb = cst.tile([P, s.TOTCOL2 + KB], BF16)
            nc.sync.dma_start(out=dstl2sb[:], in_=dstl2t[:, :])
            if has_b1:
                b1sb = cst.tile([P, HID], F32)
                nc.sync.dma_start(out=b1sb[:], in_=b1bc[:, :])
            if has_b2:
                b2sb = cst.tile([P, OUT_CH], F32)
                nc.sync.dma_start(out=b2sb[:], in_=b2bc[:, :])

            # ---------------- layer 1: aggregate xdis, emit h2 table -------
            with (
                tc.tile_pool(name="g1", bufs=BUFS_G1) as pg,
                tc.tile_pool(name="m1", bufs=BUFS_M1) as pm,
                tc.tile_pool(name="s1", bufs=3) as psb,
                tc.tile_pool(name="z1", bufs=BUFS_Z1, space="PSUM") as pz,
                tc.tile_pool(name="t1", bufs=2, space="PSUM") as pt,
            ):

                def _l1_epilogue(p, zt):
                    zsb = psb.tile([P, IN_CH], BF16, tag="zsb")
                    nc.scalar.copy(out=zsb[:], in_=zt[:])
                    zt_ts = psb.tile([P, KT * P], BF16, tag="zts")
                    for k in range(KT):
                        tr = pt.tile([P, P], BF16, tag="tr")
                        nc.tensor.transpose(
                            out=tr[:],
                            in_=zsb[:, k * P : (k + 1) * P],
                            identity=idsb[:],
                        )
                        nc.scalar.copy(
                            out=zt_ts[:, k * P : (k + 1) * P], in_=tr[:]
                        )
                    o1ps = pt.tile([P, HID], F32, tag="tp")
                    for k in range(KT):
                        nc.tensor.matmul(
                            o1ps[:],
                            lhsT=zt_ts[:, k * P : (k + 1) * P],
                            rhs=w1sb[:, k * HID : (k + 1) * HID],
                            start=(k == 0),
                            stop=(k == KT - 1),
                        )
                    o1sb = psb.tile([P, HID], BF16, tag="o1")
                    if has_b1:
                        t0f = psb.tile([P, HID], F32, tag="t0f")
                        nc.vector.tensor_scalar_mul(
                            t0f[:], o1ps[:], dissb[:, p : p + 1]
                        )
                        nc.vector.tensor_tensor(t0f[:], t0f[:], b1sb[:], op=ADD)
                        nc.scalar.activation(o1sb[:], t0f[:], RELU)
                    else:
                        nc.scalar.activation(
                            o1sb[:], o1ps[:], RELU, scale=dissb[:, p : p + 1]
                        )
                    trp = pt.tile([P, HID], BF16, tag="tr")
                    nc.tensor.transpose(out=trp[:], in_=o1sb[:], identity=idsb[:])
                    o1t = psb.tile([P, HID], BF16, tag="o1t")
                    nc.scalar.copy(out=o1t[:], in_=trp[:])
                    h2ps = pt.tile([P, P], F32, tag="tp")
                    nc.tensor.matmul(
                        h2ps[:, :OUT_CH],
                        lhsT=o1t[:],
                        rhs=w2sb[:],
                        start=True,
                        stop=True,
                    )
                    h2sb = psb.tile([P, OUT_CH], BF16, tag="h2")
                    nc.vector.tensor_scalar_mul(
                        h2sb[:], h2ps[:, :OUT_CH], dissb[:, p : p + 1]
                    )
                    nc.sync.dma_start(
                        out=h2s[p * P : (p + 1) * P, :], in_=h2sb[:]
                    )

                pending = None
                j = 0
                for g in range(NGRP):
                    gt = pg.tile([P, NCHG, IN_CH], BF16, tag="gt")
                    for q in range(NQ):
                        nq = int(s.nch[g, q])
                        if nq == 0:
                            continue
                        o = int(s.oq[g, q])
                        io = int(s.icol[g, q])
                        nidx = nq * P
                        nc.gpsimd.dma_gather(
                            gt[:, o : o + nq, :],
                            xdis[s.qbase[q] : s.qbase[q] + s.qrows[q], :],
                            esrcsb[:, io : io + nq * (P // 16)],
                            nidx,
                            nidx_regs[nidx],
                            IN_CH,
                            single_packet=False,
                        )
                    j0c = j
                    cig = 0
                    cur = None
                    for p, cols in s.groups[g]:
                        zt = pz.tile([P, IN_CH], F32, tag="z")
                        ncol = len(cols)
                        for ki, (q, ci, _t0, _t1) in enumerate(cols):
                            bb, r = divmod(cig, KB)
                            if r == 0:
                                cur = pm.tile([P, P * KB], BF16, tag="m")
                                nc.vector.tensor_tensor(
                                    cur[:].rearrange("p (l k) -> p l k", k=KB),
                                    iotaKsb[:].rearrange(
                                        "p (l k) -> p l k", k=KB
                                    ),
                                    dstlsb[:, j0c + bb * KB : j0c + (bb + 1) * KB]
                                    .unsqueeze(1)
                                    .broadcast_to([P, P, KB]),
                                    op=ISEQ,
                                )
                            cig += 1
                            j += 1
                            mv = cur[:].rearrange("p (l k) -> p k l", k=KB)
                            nc.tensor.matmul(
                                zt[:],
                                lhsT=mv[:, r, :],
                                rhs=gt[:, int(s.oq[g, q]) + ci, :],
                                start=(ki == 0),
                                stop=(ki == ncol - 1),
                            )
                        # software pipeline: emit the PREVIOUS block's dense
                        # epilogue now, so its PE ops never stall on ACT
                        # round-trips (next block's agg matmuls fill the gap).
                        if pending is not None:
                            pending()
                        pending = (lambda p=p, zt=zt: _l1_epilogue(p, zt))
                        continue

                if pending is not None:
                    pending()
                    pending = None
                assert j == s.TOTCOL

            # ---------------- AllGather of the compact h2 message table ----
            nc.gpsimd.collective_compute(
                "AllGather",
                mybir.AluOpType.bypass,
                replica_groups=rg,
                ins=[h2s[:, :]],
                outs=[h2f[:, :]],
            )

            # ---------------- layer 2: pair-gather h2, write output --------
            h2fp = h2f[:, :].rearrange("(r two) f -> r (two f)", two=2)
            with (
                tc.tile_pool(name="g2", bufs=BUFS_G2) as pg2,
                tc.tile_pool(name="m2", bufs=BUFS_M2) as pm2,
                tc.tile_pool(name="s2", bufs=3) as psb2,
                tc.tile_pool(name="z2", bufs=BUFS_Z2, space="PSUM") as pz2,
                tc.tile_pool(name="e2p", bufs=1) as pe2,
            ):
                esrc2sb = pe2.tile([P, s.TOTICOL], I16)
                nc.sync.dma_start(out=esrc2sb[:], in_=esrc2t[:, :])
                j = 0
                for g in range(NGRP):
                    gt = pg2.tile([P, NCHG, P], BF16, tag="gt")
                    for h in range(NQ // 2):
                        nq = int(s.nch[g, 2 * h] + s.nch[g, 2 * h + 1])
                        if nq == 0:
                            continue
                        o = int(s.oq[g, 2 * h])
                        io = int(s.icol2[g, h])
                        nidx = nq * P
                        nc.gpsimd.dma_gather(
                            gt[:, o : o + nq, :],
                            h2fp[h * QS : h * QS + s.q2rows[h], :],
                            esrc2sb[:, io : io + nq * (P // 16)],
                            nidx,
                            nidx_regs[nidx],
                            P,
                            single_packet=False,
                        )
                    j0c = j
                    cig = 0
                    cur = None
                    for p, cols in s.groups2[g]:
                        zt = pz2.tile([P, OUT_CH], F32, tag="z")
                        ncol = len(cols)
                        for ki, (q, ci, par, _lo, _hi) in enumerate(cols):
                            bb, r = divmod(cig, KB)
                            if r == 0:
                                cur = pm2.tile([P, P * KB], BF16, tag="m")
                                nc.vector.tensor_tensor(
                                    cur[:].rearrange("p (l k) -> p l k", k=KB),
                                    iotaKsb[:].rearrange(
                                        "p (l k) -> p l k", k=KB
                                    ),
                                    dstl2sb[
                                        :, j0c + bb * KB : j0c + (bb + 1) * KB
                                    ]
                                    .unsqueeze(1)
                                    .broadcast_to([P, P, KB]),
                                    op=ISEQ,
                                )
                            cig += 1
                            j += 1
                            mv = cur[:].rearrange("p (l k) -> p k l", k=KB)
                            nc.tensor.matmul(
                                zt[:],
                                lhsT=mv[:, r, :],
                                rhs=gt[
                                    :,
                                    int(s.oq[g, q]) + ci,
                                    par * OUT_CH : (par + 1) * OUT_CH,
                                ],
                                start=(ki == 0),
                                stop=(ki == ncol - 1),
                            )
                        osb = psb2.tile([P, OUT_CH], F32, tag="o")
                        if has_b2:
                            t2f = psb2.tile([P, OUT_CH], F32, tag="t2f")
                            nc.vector.tensor_scalar_mul(
                                t2f[:], zt[:], dissb[:, p : p + 1]
                            )
                            nc.vector.tensor_tensor(t2f[:], t2f[:], b2sb[:], op=ADD)
                            nc.scalar.activation(osb[:], t2f[:], RELU)
                        else:
                            nc.scalar.activation(
                                osb[:], zt[:], RELU, scale=dissb[:, p : p + 1]
                            )
                        nc.sync.dma_start(
                            out=outY[p * P : (p + 1) * P, :], in_=osb[:]
                        )
                assert j == s.TOTCOL2

    _spill_waits(nc)
    mybir.codegen_inst_isa_subclasses(nc)
    return nc


# --------------------------------------------------------------------------
# Entry point
# --------------------------------------------------------------------------


def kernel(x, edge_index, W1, b1, W2, b2):
    x = np.asarray(x, dtype=np.float32)
    W1 = np.asarray(W1, dtype=np.float32)
    W2 = np.asarray(W2, dtype=np.float32)
    b1 = np.asarray(b1, dtype=np.float32)
    b2 = np.asarray(b2, dtype=np.float32)
    src = np.asarray(edge_index[0]).astype(np.int64)
    dst = np.asarray(edge_index[1]).astype(np.int64)

    N, IN_CH = x.shape
    HID = W1.shape[1]
    OUT_CH = W2.shape[1]

    s = _prepare(N, src, dst)

    # full xdis replica, pgid-indexed (empty gids = 0)
    xdis = np.zeros((s.NTOT, IN_CH), ml_dtypes.bfloat16)
    xdis[s.gid_of] = (x * s.dis[:, None]).astype(ml_dtypes.bfloat16)

    W1b = np.ascontiguousarray(W1.astype(ml_dtypes.bfloat16))
    W2b = np.ascontiguousarray(W2.astype(ml_dtypes.bfloat16))
    has_b1 = bool(np.any(b1))
    has_b2 = bool(np.any(b2))
    b1bc = np.ascontiguousarray(np.broadcast_to(b1, (P, HID)).astype(np.float32))
    b2bc = np.ascontiguousarray(np.broadcast_to(b2, (P, OUT_CH)).astype(np.float32))

    nc = _build_program(s, IN_CH, HID, OUT_CH, has_b1, has_b2)

    iotaK = np.zeros((P, P * KB), ml_dtypes.bfloat16)
    for l in range(P):
        iotaK[:, l * KB : (l + 1) * KB] = l

    in_maps = []
    node_maps = []
    for c in range(NCORES):
        esrc16, esrc2, dstlc, dstl2c, disTc, nodes = _core_tensors(s, c)
        node_maps.append(nodes)
        in_maps.append(
            {
                "xdis": xdis,
                "W1": W1b,
                "W2c": W2b,
                "b1bc": b1bc,
                "b2bc": b2bc,
                "disT": disTc,
                "iotaK": iotaK,
                "esrc": esrc16,
                "esrc2": esrc2,
                "dstl": dstlc,
                "dstl2": dstl2c,
            }
        )

    res = run_bass_kernel_spmd(nc, in_maps, core_ids=list(range(NCORES)))
    global _last_results, _last_nc
    _last_results = res
    _last_nc = nc

    out = np.empty((N, OUT_CH), np.float32)
    for c in range(NCORES):
        oc = res.results[c]["outY"]  # [NPC, OUT_CH], row = p*128+lane
        nodes = node_maps[c].reshape(-1)  # [NB*P] original node or -1
        valid = nodes >= 0
        out[nodes[valid]] = oc[valid]
    return out


# revision 15
# speedup vs baseline: 3.3191x; 1.0283x over previous
"""2-layer GCN (PyG GCNConv x2 + ReLU) on 8 Trainium2 NeuronCores.

Math per layer (A from edge_index, deg = indeg(dst)+1, dis = deg^-1/2):
    out[d] = relu( dis_d * ( sum_{e: s->d} dis_s*h[s] + dis_d*h[d] ) @ W + b )

Strategy:
  * Layer 1 is aggregate-then-transform: every core holds a full replica of
    xdis = x*dis (bf16) in its HBM (NO layer-1 collective).  Each core
    aggregates Z = sum xdis[src] for its own destination blocks with large
    dma_gather calls + one-hot-mask matmuls, then applies W1 and W2 densely
    per 128-node block, producing the layer-2 message table h2 = dis*(relu@W2).
  * One AllGather shares the compact [NTOT, 64] bf16 h2 table.
  * Layer 2 gathers PAIRS of h2 rows (256B descriptors cover two nodes ->
    half the per-edge DMA cost); slots are parity-sorted inside each
    (block, slice) segment so each mask column selects one 64-col half.
  * Self-loops are folded in as ordinary edges (src == dst).
  * dma_gather uses int16 indices: the node table is split in 4 row slices
    of <=32768 (layer 1) / 2 pair-slices (layer 2).  The slot schedule is
    made identical across cores by padding each (block, slice, parity)
    sub-segment to the max over cores; only tensor data differs per core.
  * One-hot masks are built 8 columns per DVE instruction (interleaved iota
    constant + broadcast-AP tensor_tensor is_equal); the PE reads the mask
    slices with a strided lhsT access pattern.
"""

import math

import ml_dtypes
import numpy as np

import concourse.bass as bass
import concourse.mybir as mybir
import concourse.tile as tile
from concourse import library_config
from concourse.bass_utils import run_bass_kernel_spmd
from concourse.masks import make_identity
from concourse.vector_clock import ScopedClock

P = 128
NCORES = 8
PAD_LANE = 1000.0  # dst-lane sentinel for padding edge slots (one-hot = 0)
QS = 32768  # dma_gather int16 index range -> table row-slice size
GRP = 7  # dst blocks per gather group
KB = 8  # one-hot mask columns built per DVE instruction
BUFS_G1, BUFS_M1, BUFS_Z1 = 2, 6, 3
BUFS_G2, BUFS_M2, BUFS_Z2 = 2, 6, 6

F32 = mybir.dt.float32
BF16 = mybir.dt.bfloat16
I16 = mybir.dt.int16


def _patched_drain_and_barrier(self, tick_clock, wait_clock):
    # This walrus build rejects >1 sem wait on TPB_CTRL (Drain) instructions.
    # Spill the tile-epilogue drain waits onto extra single-wait drains.
    drain_inst = self.nc.sync.drain()
    wait_clock.add_sem_waits(
        drain_inst.ins, ScopedClock({None: tick_clock.global_clock})
    )
    si = drain_inst.ins.sync_info
    waits = list(si.on_wait)
    if len(waits) > 1:
        while len(si.on_wait):
            si.on_wait.pop()
        si.on_wait.append(waits[0])
        for w in waits[1:]:
            d2 = self.nc.sync.drain(fusable=False)
            si2 = d2.ins.sync_info
            if si2 is None:
                d2.ins.sync_info = mybir.SyncInfo(on_wait=[w], on_update=[])
            else:
                si2.on_wait.append(w)
    self.nc.all_engine_barrier()
    popped = self.nc._tile_sem_poison_stack.pop()
    assert popped is self._sem_poison
    self.nc.clear_and_free_semaphores(list(self.sems.allocated().values()))
    self.nc.all_engine_barrier()


tile.TileContext._drain_and_barrier = _patched_drain_and_barrier


def _spill_waits(nc, max_waits=1):
    """This walrus build accepts at most one sync wait per instruction.
    Move extra waits onto dedicated single-wait NoOps ahead of the
    instruction on the same engine (engines execute in program order)."""
    n = 0
    for f in nc.m.functions:
        for blk in f.blocks:
            il = blk.instructions
            out = []
            for inst in il:
                si = inst.sync_info
                if si is not None and len(si.on_wait) > max_waits:
                    waits = list(si.on_wait)
                    while len(si.on_wait):
                        si.on_wait.pop()
                    for w in waits[:max_waits]:
                        si.on_wait.append(w)
                    for w in waits[max_waits:]:
                        nop = mybir.InstNoOp(
                            name=f"waitspill-{n}",
                            sync_info=mybir.SyncInfo(on_wait=[w], on_update=[]),
                            bass_nofuse=True,
                            engine=inst.engine,
                        )
                        n += 1
                        out.append(nop)
                out.append(inst)
            blk.instructions = out
    return n


# --------------------------------------------------------------------------
# Host-side schedule construction
# --------------------------------------------------------------------------


class _Sched:
    pass


def _prepare(N, src, dst):
    """Node->gid assignment, slot buckets, and the (core-uniform) gather /
    column schedule for both layers."""
    s = _Sched()
    NB = int(math.ceil(N / (NCORES * P)))
    TB = NB * NCORES
    NPC = NB * P
    NTOT = TB * P
    NGRP = int(math.ceil(NB / GRP))
    assert NGRP * GRP == NB, (NB, NGRP)
    NQ = int(math.ceil(NTOT / QS))
    assert NQ % 2 == 0
    qbase = [q * QS for q in range(NQ)]
    qrows = [min(QS, NTOT - q * QS) for q in range(NQ)]
    # layer-2 pair slices: pair row = pgid >> 1, slice h covers q in (2h, 2h+1)
    q2rows = [
        (qrows[2 * h] + qrows[2 * h + 1] + 1) // 2 for h in range(NQ // 2)
    ]

    indeg = np.bincount(dst, minlength=N).astype(np.int64)
    dis = (1.0 / np.sqrt(indeg.astype(np.float64) + 1.0)).astype(np.float32)

    # snake assignment of degree-sorted nodes over TB blocks
    order = np.argsort(-indeg, kind="stable")
    i = np.arange(N)
    rnd = i // TB
    pos = i % TB
    blk_i = np.where(rnd % 2 == 0, pos, TB - 1 - pos)
    assert rnd.max() < P
    gid_of = np.empty(N, np.int64)
    gid_of[order] = blk_i * P + rnd

    # balance blocks over cores by slot count, then renumber into PHYSICAL
    # gid space: pgid = (core*NB + position)*128 + lane.  The AllGathered h2f
    # table is laid out in physical order, so gather indices, slice
    # bucketing, the xdis table, and outputs all use pgid.
    gdst0 = gid_of[dst] >> 7
    btot = np.bincount(gdst0, minlength=TB) + np.bincount(
        gid_of >> 7, minlength=TB
    )
    bord = np.argsort(-btot, kind="stable")
    block_of0 = np.empty((NCORES, NB), np.int64)
    for p in range(NB):
        row = bord[p * NCORES : (p + 1) * NCORES]
        if p % 2:
            row = row[::-1]
        block_of0[:, p] = row
    phys_pos = np.empty(TB, np.int64)
    for c in range(NCORES):
        for p in range(NB):
            phys_pos[block_of0[c, p]] = c * NB + p
    gid_of = phys_pos[gid_of >> 7] * P + (gid_of & 127)
    node_of_gid = np.full(NTOT, -1, np.int64)
    node_of_gid[gid_of] = np.arange(N)

    # slots = edges + self-loops, bucketed by (dst block, src slice, parity)
    all_src = np.concatenate([gid_of[src], gid_of])
    all_dst = np.concatenate([gid_of[dst], gid_of])
    sblk = all_dst >> 7
    slane = (all_dst & 127).astype(np.float32)
    sq = all_src // QS
    key2 = (sblk * NQ + sq) * 2 + (all_src & 1)
    ord2 = np.argsort(key2, kind="stable")
    k_src = all_src[ord2]
    k_lane = slane[ord2]
    cnt2 = np.bincount(key2, minlength=TB * NQ * 2).reshape(TB, NQ, 2)
    offs2 = np.zeros(TB * NQ * 2 + 1, np.int64)
    np.cumsum(cnt2.reshape(-1), out=offs2[1:])
    block_of = (np.arange(NCORES)[:, None] * NB) + np.arange(NB)[None, :]

    # uniform per-(position, slice, parity) sub-segment lengths
    seg_ev = cnt2[block_of, :, 0].max(axis=0)  # [NB, NQ]
    seg_od = cnt2[block_of, :, 1].max(axis=0)
    seg_len = seg_ev + seg_od

    # per-group call sizes + column schedules
    nch = np.zeros((NGRP, NQ), np.int64)
    oq = np.zeros((NGRP, NQ), np.int64)  # chunk offset of call q in group tile
    groups = []  # L1: per group, list of (p, [(q, ci, lo, hi)])
    groups2 = []  # L2: per group, list of (p, [(q, ci, par, lo, hi)])
    for g in range(NGRP):
        ps = list(range(g * GRP, (g + 1) * GRP))
        for q in range(NQ):
            L = int(seg_len[ps, q].sum())
            nch[g, q] = (L + P - 1) // P
        oq[g] = np.concatenate([[0], np.cumsum(nch[g])[:-1]])
        blocks = []
        blocks2 = []
        for bi, p in enumerate(ps):
            cols = []
            cols2 = []
            for q in range(NQ):
                t0 = int(seg_len[ps[:bi], q].sum())
                t1 = t0 + int(seg_len[p, q])
                if t1 == t0:
                    continue
                for ci in range(t0 // P, (t1 - 1) // P + 1):
                    cols.append((q, ci, t0, t1))
                for par, (lo, hi) in enumerate(
                    [
                        (t0, t0 + int(seg_ev[p, q])),
                        (t0 + int(seg_ev[p, q]), t1),
                    ]
                ):
                    if hi == lo:
                        continue
                    for ci in range(lo // P, (hi - 1) // P + 1):
                        cols2.append((q, ci, par, lo, hi))
            assert cols and cols2
            blocks.append((p, cols))
            blocks2.append((p, cols2))
        groups.append(blocks)
        groups2.append(blocks2)
    NCHG = int(nch.sum(1).max())
    TOTCOL = sum(len(c) for blks in groups for _, c in blks)
    TOTCOL2 = sum(len(c) for blks in groups2 for _, c in blks)

    # int16 idx column offsets: layer 1 per (g, q); layer 2 per (g, half)
    icol = np.zeros((NGRP, NQ), np.int64)
    run = 0
    for g in range(NGRP):
        for q in range(NQ):
            icol[g, q] = run
            run += int(nch[g, q]) * (P // 16)
    TOTICOL = int(run)
    icol2 = np.zeros((NGRP, NQ // 2), np.int64)
    run = 0
    for g in range(NGRP):
        for h in range(NQ // 2):
            icol2[g, h] = run
            run += int(nch[g, 2 * h] + nch[g, 2 * h + 1]) * (P // 16)
    assert int(run) == TOTICOL

    s.NB, s.TB, s.NPC, s.NTOT, s.NGRP, s.NQ = NB, TB, NPC, NTOT, NGRP, NQ
    s.qbase, s.qrows, s.q2rows = qbase, qrows, q2rows
    s.dis, s.gid_of, s.node_of_gid = dis, gid_of, node_of_gid
    s.k_src, s.k_lane, s.cnt2, s.offs2 = k_src, k_lane, cnt2, offs2
    s.block_of, s.seg_len, s.seg_ev = block_of, seg_len, seg_ev
    s.nch, s.oq, s.NCHG = nch, oq, NCHG
    s.groups, s.TOTCOL = groups, TOTCOL
    s.groups2, s.TOTCOL2 = groups2, TOTCOL2
    s.icol, s.icol2, s.TOTICOL = icol, icol2, TOTICOL
    return s


def _core_tensors(s, c):
    """Per-core dstl / dstl2 / esrc / esrc2 / disT arrays."""
    NB, NQ = s.NB, s.NQ

    esrc16 = np.zeros((P, s.TOTICOL), np.int16)
    esrc2 = np.zeros((P, s.TOTICOL), np.int16)
    dstl = np.full((P, s.TOTCOL + KB), PAD_LANE, np.float32)
    dstl2 = np.full((P, s.TOTCOL2 + KB), PAD_LANE, np.float32)

    # per (position, q): padded slot arrays (absolute src pgid + lanes)
    seg_src = {}
    seg_lane = {}
    for p in range(NB):
        tb = s.block_of[c, p]
        for q in range(NQ):
            L = int(s.seg_len[p, q])
            sv = np.full(L, s.qbase[q], np.int64)
            lv = np.full(L, PAD_LANE, np.float32)
            o0 = s.offs2[(tb * NQ + q) * 2]
            ne = int(s.cnt2[tb, q, 0])
            sv[:ne] = s.k_src[o0 : o0 + ne]
            lv[:ne] = s.k_lane[o0 : o0 + ne]
            o1 = s.offs2[(tb * NQ + q) * 2 + 1]
            no = int(s.cnt2[tb, q, 1])
            se = int(s.seg_ev[p, q])
            sv[se : se + no] = s.k_src[o1 : o1 + no]
            lv[se : se + no] = s.k_lane[o1 : o1 + no]
            seg_src[p, q] = sv
            seg_lane[p, q] = lv

    def wrap(seq):
        return np.tile(seq.reshape(-1, 16).T, (8, 1))

    j = 0
    j2 = 0
    for g in range(s.NGRP):
        ps = list(range(g * GRP, (g + 1) * GRP))
        # absolute src values per layer-1 call (incl. all pads)
        absq = {}
        for q in range(NQ):
            L = int(s.nch[g, q]) * P
            seq = np.full(L, s.qbase[q], np.int64)
            t = 0
            for p in ps:
                n = int(s.seg_len[p, q])
                seq[t : t + n] = seg_src[p, q]
                t += n
            absq[q] = seq
            io = s.icol[g, q]
            esrc16[:, io : io + L // 16] = wrap(
                (seq - s.qbase[q]).astype(np.int16)
            )
        for h in range(NQ // 2):
            seq = np.concatenate([absq[2 * h], absq[2 * h + 1]])
            io = s.icol2[g, h]
            esrc2[:, io : io + len(seq) // 16] = wrap(
                ((seq >> 1) - h * QS).astype(np.int16)
            )
        # mask columns (emission order: block-major within group)
        for p, cols in s.groups[g]:
            for q, ci, t0, _t1 in cols:
                lv = seg_lane[p, q]
                lo = max(ci * P, t0)
                hi = min(ci * P + P, t0 + len(lv))
                if hi > lo:
                    dstl[lo - ci * P : hi - ci * P, j] = lv[lo - t0 : hi - t0]
                j += 1
        for p, cols in s.groups2[g]:
            for q, ci, par, lo_s, hi_s in cols:
                lv = seg_lane[p, q]
                t0 = lo_s if par == 0 else lo_s - int(s.seg_ev[p, q])
                lo = max(ci * P, lo_s)
                hi = min(ci * P + P, hi_s)
                if hi > lo:
                    dstl2[lo - ci * P : hi - ci * P, j2] = lv[
                        lo - t0 : hi - t0
                    ]
                j2 += 1
    assert j == s.TOTCOL and j2 == s.TOTCOL2

    gids = s.block_of[c][:, None] * P + np.arange(P)[None, :]  # [NB, P]
    nodes = s.node_of_gid[gids]
    disT = np.ones((P, NB), np.float32)
    valid = nodes >= 0
    disT.T[valid] = s.dis[nodes[valid]]

    return (
        np.ascontiguousarray(esrc16),
        np.ascontiguousarray(esrc2),
        np.ascontiguousarray(dstl.astype(ml_dtypes.bfloat16)),
        np.ascontiguousarray(dstl2.astype(ml_dtypes.bfloat16)),
        np.ascontiguousarray(disT),
        nodes,
    )


# --------------------------------------------------------------------------
# Device program
# --------------------------------------------------------------------------


def _build_program(s, IN_CH, HID, OUT_CH, has_b1, has_b2):
    NB, NQ, NCHG, NGRP = s.NB, s.NQ, s.NCHG, s.NGRP
    KT = IN_CH // P
    assert IN_CH % P == 0 and HID == P and OUT_CH * 2 == P

    nc = bass.Bass()
    xdis = nc.dram_tensor("xdis", [s.NTOT, IN_CH], BF16, kind="ExternalInput")
    W1 = nc.dram_tensor("W1", [IN_CH, HID], BF16, kind="ExternalInput")
    W2c = nc.dram_tensor("W2c", [HID, OUT_CH], BF16, kind="ExternalInput")
    b1bc = nc.dram_tensor("b1bc", [P, HID], F32, kind="ExternalInput")
    b2bc = nc.dram_tensor("b2bc", [P, OUT_CH], F32, kind="ExternalInput")
    disT = nc.dram_tensor("disT", [P, NB], F32, kind="ExternalInput")
    iotaK = nc.dram_tensor("iotaK", [P, P * KB], BF16, kind="ExternalInput")
    esrc = nc.dram_tensor("esrc", [P, s.TOTICOL], I16, kind="ExternalInput")
    esrc2t = nc.dram_tensor("esrc2", [P, s.TOTICOL], I16, kind="ExternalInput")
    dstl = nc.dram_tensor("dstl", [P, s.TOTCOL + KB], BF16, kind="ExternalInput")
    dstl2t = nc.dram_tensor(
        "dstl2", [P, s.TOTCOL2 + KB], BF16, kind="ExternalInput"
    )
    outY = nc.dram_tensor("outY", [s.NPC, OUT_CH], F32, kind="ExternalOutput")

    h2s = nc.dram_tensor("h2s", [s.NPC, OUT_CH], BF16)
    h2f = nc.dram_tensor("h2f", [s.NTOT, OUT_CH], BF16)

    rg = [list(range(NCORES))]
    RELU = mybir.ActivationFunctionType.Relu
    ADD = mybir.AluOpType.add
    ISEQ = mybir.AluOpType.is_equal

    with tile.TileContext(nc) as tc:
        with tc.tile_pool(name="const", bufs=1) as cst:
            iotaKsb = cst.tile([P, P * KB], BF16)
            nc.sync.dma_start(out=iotaKsb[:], in_=iotaK[:, :])
            idsb = cst.tile([P, P], BF16)
            make_identity(nc, idsb[:])
            # custom-op ucode (dma_gather) — after the stock gpsimd ops above
            nc.gpsimd.load_library(library_config.mlp)
            nvals = {int(v) * P for v in np.unique(s.nch) if v}
            nvals |= {
                int(s.nch[g, 2 * h] + s.nch[g, 2 * h + 1]) * P
                for g in range(NGRP)
                for h in range(NQ // 2)
            }
            nidx_regs = {}
            for v in sorted(nvals):
                nidx_regs[v] = nc.gpsimd.to_reg(v)

            w1sb = cst.tile([P, KT * HID], BF16)
            for k in range(KT):
                nc.sync.dma_start(
                    out=w1sb[:, k * HID : (k + 1) * HID],
                    in_=W1[k * P : (k + 1) * P, :],
                )
            w2sb = cst.tile([P, OUT_CH], BF16)
            nc.sync.dma_start(out=w2sb[:], in_=W2c[:, :])
            dissb = cst.tile([P, NB], F32)
            nc.sync.dma_start(out=dissb[:], in_=disT[:, :])
            esrcsb = cst.tile([P, s.TOTICOL], I16)
            nc.sync.dma_start(out=esrcsb[:], in_=esrc[:, :])
            dstlsb = cst.tile([P, s.TOTCOL + KB], BF16)
            nc.sync.dma_start(out=dstlsb[:], in_=dstl[:, :])
            dstl2sb = cst.tile([P, s.TOTCOL2 + KB], BF16)
            nc.sync.dma_start(out=dstl2sb[:], in_=dstl2t[:, :])
            if has_b1:
                b1sb = cst.tile([P, HID], F32)
                nc.sync.dma_start(out=b1sb[:], in_=b1bc[:, :])
            if has_b2:
                b2sb = cst.tile([P, OUT_CH], F32)
                nc.sync.dma_start(out=b2sb[:], in_=b2bc[:, :])

            # ---------------- layer 1: aggregate xdis, emit h2 table -------
            with (
                tc.tile_pool(name="g1", bufs=BUFS_G1) as pg,
                tc.tile_pool(name="m1", bufs=BUFS_M1) as pm,
                tc.tile_pool(name="s1", bufs=3) as psb,
                tc.tile_pool(name="z1", bufs=BUFS_Z1, space="PSUM") as pz,
                tc.tile_pool(name="t1", bufs=2, space="PSUM") as pt,
            ):

                def _l1_epi_a(p, zt):
                    zsb = psb.tile([P, IN_CH], BF16, tag="zsb")
                    nc.scalar.copy(out=zsb[:], in_=zt[:])
                    zt_ts = psb.tile([P, KT * P], BF16, tag="zts")
                    for k in range(KT):
                        tr = pt.tile([P, P], BF16, tag="tr")
                        nc.tensor.transpose(
                            out=tr[:],
                            in_=zsb[:, k * P : (k + 1) * P],
                            identity=idsb[:],
                        )
                        nc.scalar.copy(
                            out=zt_ts[:, k * P : (k + 1) * P], in_=tr[:]
                        )
                    return zt_ts

                def _l1_epi_b(p, zt_ts):
                    o1ps = pt.tile([P, HID], F32, tag="tp")
                    for k in range(KT):
                        nc.tensor.matmul(
                            o1ps[:],
                            lhsT=zt_ts[:, k * P : (k + 1) * P],
                            rhs=w1sb[:, k * HID : (k + 1) * HID],
                            start=(k == 0),
                            stop=(k == KT - 1),
                        )
                    o1sb = psb.tile([P, HID], BF16, tag="o1")
                    if has_b1:
                        t0f = psb.tile([P, HID], F32, tag="t0f")
                        nc.vector.tensor_scalar_mul(
                            t0f[:], o1ps[:], dissb[:, p : p + 1]
                        )
                        nc.vector.tensor_tensor(t0f[:], t0f[:], b1sb[:], op=ADD)
                        nc.scalar.activation(o1sb[:], t0f[:], RELU)
                    else:
                        nc.scalar.activation(
                            o1sb[:], o1ps[:], RELU, scale=dissb[:, p : p + 1]
                        )
                    trp = pt.tile([P, HID], BF16, tag="tr")
                    nc.tensor.transpose(out=trp[:], in_=o1sb[:], identity=idsb[:])
                    o1t = psb.tile([P, HID], BF16, tag="o1t")
                    nc.scalar.copy(out=o1t[:], in_=trp[:])
                    h2ps = pt.tile([P, P], F32, tag="tp")
                    nc.tensor.matmul(
                        h2ps[:, :OUT_CH],
                        lhsT=o1t[:],
                        rhs=w2sb[:],
                        start=True,
                        stop=True,
                    )
                    h2sb = psb.tile([P, OUT_CH], BF16, tag="h2")
                    nc.vector.tensor_scalar_mul(
                        h2sb[:], h2ps[:, :OUT_CH], dissb[:, p : p + 1]
                    )
                    nc.sync.dma_start(
                        out=h2s[p * P : (p + 1) * P, :], in_=h2sb[:]
                    )

                pend_a = None
                pend_b = None
                j = 0
                for g in range(NGRP):
                    gt = pg.tile([P, NCHG, IN_CH], BF16, tag="gt")
                    for q in range(NQ):
                        nq = int(s.nch[g, q])
                        if nq == 0:
                            continue
                        o = int(s.oq[g, q])
                        io = int(s.icol[g, q])
                        nidx = nq * P
                        nc.gpsimd.dma_gather(
                            gt[:, o : o + nq, :],
                            xdis[s.qbase[q] : s.qbase[q] + s.qrows[q], :],
                            esrcsb[:, io : io + nq * (P // 16)],
                            nidx,
                            nidx_regs[nidx],
                            IN_CH,
                            single_packet=False,
                        )
                    j0c = j
                    cig = 0
                    cur = None
                    for p, cols in s.groups[g]:
                        zt = pz.tile([P, IN_CH], F32, tag="z")
                        ncol = len(cols)
                        for ki, (q, ci, _t0, _t1) in enumerate(cols):
                            bb, r = divmod(cig, KB)
                            if r == 0:
                                cur = pm.tile([P, P * KB], BF16, tag="m")
                                nc.vector.tensor_tensor(
                                    cur[:].rearrange("p (l k) -> p l k", k=KB),
                                    iotaKsb[:].rearrange(
                                        "p (l k) -> p l k", k=KB
                                    ),
                                    dstlsb[:, j0c + bb * KB : j0c + (bb + 1) * KB]
                                    .unsqueeze(1)
                                    .broadcast_to([P, P, KB]),
                                    op=ISEQ,
                                )
                            cig += 1
                            j += 1
                            mv = cur[:].rearrange("p (l k) -> p k l", k=KB)
                            nc.tensor.matmul(
                                zt[:],
                                lhsT=mv[:, r, :],
                                rhs=gt[:, int(s.oq[g, q]) + ci, :],
                                start=(ki == 0),
                                stop=(ki == ncol - 1),
                            )
                        # 2-stage software pipeline: after agg(p), emit
                        # stage B of block p-2 then stage A of block p-1, so
                        # each stage's PE ops never stall on the ACT hops of
                        # the same block (agg matmuls fill the gaps).
                        if pend_b is not None:
                            pend_b()
                            pend_b = None
                        if pend_a is not None:
                            pa, zts = pend_a[0], pend_a[1]()
                            pend_b = (lambda pa=pa, zts=zts: _l1_epi_b(pa, zts))
                            pend_a = None
                        pend_a = (p, (lambda p=p, zt=zt: _l1_epi_a(p, zt)))
                        continue

                if pend_b is not None:
                    pend_b()
                if pend_a is not None:
                    pa, zts = pend_a[0], pend_a[1]()
                    _l1_epi_b(pa, zts)
                assert j == s.TOTCOL

            # ---------------- AllGather of the compact h2 message table ----
            nc.gpsimd.collective_compute(
                "AllGather",
                mybir.AluOpType.bypass,
                replica_groups=rg,
                ins=[h2s[:, :]],
                outs=[h2f[:, :]],
            )

            # ---------------- layer 2: pair-gather h2, write output --------
            h2fp = h2f[:, :].rearrange("(r two) f -> r (two f)", two=2)
            with (
                tc.tile_pool(name="g2", bufs=BUFS_G2) as pg2,
                tc.tile_pool(name="m2", bufs=BUFS_M2) as pm2,
                tc.tile_pool(name="s2", bufs=3) as psb2,
                tc.tile_pool(name="z2", bufs=BUFS_Z2, space="PSUM") as pz2,
                tc.tile_pool(name="e2p", bufs=1) as pe2,
            ):
                esrc2sb = pe2.tile([P, s.TOTICOL], I16)
                nc.sync.dma_start(out=esrc2sb[:], in_=esrc2t[:, :])
                j = 0
                for g in range(NGRP):
                    gt = pg2.tile([P, NCHG, P], BF16, tag="gt")
                    for h in range(NQ // 2):
                        nq = int(s.nch[g, 2 * h] + s.nch[g, 2 * h + 1])
                        if nq == 0:
                            continue
                        o = int(s.oq[g, 2 * h])
                        io = int(s.icol2[g, h])
                        nidx = nq * P
                        nc.gpsimd.dma_gather(
                            gt[:, o : o + nq, :],
                            h2fp[h * QS : h * QS + s.q2rows[h], :],
                            esrc2sb[:, io : io + nq * (P // 16)],
                            nidx,
                            nidx_regs[nidx],
                            P,
                            single_packet=False,
                        )
                    j0c = j
                    cig = 0
                    cur = None
                    for p, cols in s.groups2[g]:
                        zt = pz2.tile([P, OUT_CH], F32, tag="z")
                        ncol = len(cols)
                        for ki, (q, ci, par, _lo, _hi) in enumerate(cols):
                            bb, r = divmod(cig, KB)
                            if r == 0:
                                cur = pm2.tile([P, P * KB], BF16, tag="m")
                                nc.vector.tensor_tensor(
                                    cur[:].rearrange("p (l k) -> p l k", k=KB),
                                    iotaKsb[:].rearrange(
                                        "p (l k) -> p l k", k=KB
                                    ),
                                    dstl2sb[
                                        :, j0c + bb * KB : j0c + (bb + 1) * KB
                                    ]
                                    .unsqueeze(1)
                                    .broadcast_to([P, P, KB]),
                                    op=ISEQ,
                                )
                            cig += 1
                            j += 1
                            mv = cur[:].rearrange("p (l k) -> p k l", k=KB)
                            nc.tensor.matmul(
                                zt[:],
                                lhsT=mv[:, r, :],
                                rhs=gt[
                                    :,
                                    int(s.oq[g, q]) + ci,
                                    par * OUT_CH : (par + 1) * OUT_CH,
                                ],
                                start=(ki == 0),
                                stop=(ki == ncol - 1),
                            )
                        osb = psb2.tile([P, OUT_CH], F32, tag="o")
                        if has_b2:
                            t2f = psb2.tile([P, OUT_CH], F32, tag="t2f")
                            nc.vector.tensor_scalar_mul(
                                t2f[:], zt[:], dissb[:, p : p + 1]
                            )
                            nc.vector.tensor_tensor(t2f[:], t2f[:], b2sb[:], op=ADD)
                            nc.scalar.activation(osb[:], t2f[:], RELU)
                        else:
                            nc.scalar.activation(
                                osb[:], zt[:], RELU, scale=dissb[:, p : p + 1]
                            )
                        nc.sync.dma_start(
                            out=outY[p * P : (p + 1) * P, :], in_=osb[:]
                        )
                assert j == s.TOTCOL2

    _spill_waits(nc)
    mybir.codegen_inst_isa_subclasses(nc)
    return nc


# --------------------------------------------------------------------------
# Entry point
# --------------------------------------------------------------------------


def kernel(x, edge_index, W1, b1, W2, b2):
    x = np.asarray(x, dtype=np.float32)
    W1 = np.asarray(W1, dtype=np.float32)
    W2 = np.asarray(W2, dtype=np.float32)
    b1 = np.asarray(b1, dtype=np.float32)
    b2 = np.asarray(b2, dtype=np.float32)
    src = np.asarray(edge_index[0]).astype(np.int64)
    dst = np.asarray(edge_index[1]).astype(np.int64)

    N, IN_CH = x.shape
    HID = W1.shape[1]
    OUT_CH = W2.shape[1]

    s = _prepare(N, src, dst)

    # full xdis replica, pgid-indexed (empty gids = 0)
    xdis = np.zeros((s.NTOT, IN_CH), ml_dtypes.bfloat16)
    xdis[s.gid_of] = (x * s.dis[:, None]).astype(ml_dtypes.bfloat16)

    W1b = np.ascontiguousarray(W1.astype(ml_dtypes.bfloat16))
    W2b = np.ascontiguousarray(W2.astype(ml_dtypes.bfloat16))
    has_b1 = bool(np.any(b1))
    has_b2 = bool(np.any(b2))
    b1bc = np.ascontiguousarray(np.broadcast_to(b1, (P, HID)).astype(np.float32))
    b2bc = np.ascontiguousarray(np.broadcast_to(b2, (P, OUT_CH)).astype(np.float32))

    nc = _build_program(s, IN_CH, HID, OUT_CH, has_b1, has_b2)

    iotaK = np.zeros((P, P * KB), ml_dtypes.bfloat16)
    for l in range(P):
        iotaK[:, l * KB : (l + 1) * KB] = l

    in_maps = []
    node_maps = []
    for c in range(NCORES):
        esrc16, esrc2, dstlc, dstl2c, disTc, nodes = _core_tensors(s, c)
        node_maps.append(nodes)
        in_maps.append(
            {
                "xdis": xdis,
                "W1": W1b,
                "W2c": W2b,
                "b1bc": b1bc,
                "b2bc": b2bc,
                "disT": disTc,
                "iotaK": iotaK,
                "esrc": esrc16,
                "esrc2": esrc2,
                "dstl": dstlc,
                "dstl2": dstl2c,
            }
        )

    res = run_bass_kernel_spmd(nc, in_maps, core_ids=list(range(NCORES)))
    global _last_results, _last_nc
    _last_results = res
    _last_nc = nc

    out = np.empty((N, OUT_CH), np.float32)
    for c in range(NCORES):
        oc = res.results[c]["outY"]  # [NPC, OUT_CH], row = p*128+lane
        nodes = node_maps[c].reshape(-1)  # [NB*P] original node or -1
        valid = nodes >= 0
        out[nodes[valid]] = oc[valid]
    return out
